# revision 21
# baseline (speedup 1.0000x reference)
"""Bass/Trainium2 kernel for nn_HCTargetAwareAttnNP.

Sharding: 8 cores = B(4) x Nt-half(2). Each core handles one batch b and 64
of the 128 targets, with full R_ctx[b]/phi_c[b] local (softmax over Nc stays
on-core, no collectives).

Host-path design (the wall-clock bottleneck, not device FLOPs):
- All weights are baked into the NEFF as Const tensors (nc.inline_tensor),
  keyed by a hash of the weight inputs -- so per-call PJRT traffic is just
  two small activation tensors per core (~350 KB) instead of ~5 MB of
  replicated weights per core.
- The jax persistent compilation cache is enabled so the per-call
  jax.jit(shard_map(...)) inside run_bass_kernel_spmd deserializes the
  compiled executable instead of re-running XLA/neuronx-cc.

Device layout: everything FEATURE-MAJOR (feature dim on SBUF partitions,
context positions on the free dim); weight matrices are used in native
(in x out) layout as the PE stationary operand, and the pairwise (Nc x D)
tensors per (b,t) are built directly in PSUM by accumulating matmuls. Two
targets per supertile (free dim 512 = 2 x Nc).
"""

import hashlib
import os
import numpy as np
import ml_dtypes
from contextlib import ExitStack

BF16NP = np.float16

import jax

for _k, _v in (
    ("jax_compilation_cache_dir", "/tmp/bass_jax_pcc"),
    ("jax_persistent_cache_min_compile_time_secs", 0.0),
    ("jax_persistent_cache_min_entry_size_bytes", 0),
    # source locations otherwise leak the caller's filename/lineno into the
    # MLIR module, so the persistent-cache key would differ per caller script
    ("jax_include_full_tracebacks_in_locations", False),
    ("jax_traceback_in_locations_limit", 0),
):
    try:
        jax.config.update(_k, _v)
    except Exception:
        pass

import concourse.bass as bass
import concourse.tile as tile
from concourse import bacc, mybir
from concourse.bass_utils import run_bass_kernel_spmd

F32 = mybir.dt.float32
F32R = mybir.dt.float32r
BF16 = mybir.dt.float16
AF = mybir.ActivationFunctionType
ALU = mybir.AluOpType

B, NT_FULL, NC, D, DPHI, HID, H, DK = 4, 128, 256, 256, 16, 128, 8, 32
NCORES = 8
NT = 64                         # local targets per core (half of Nt)
ST_T = 2                        # targets per supertile
C2 = ST_T * NC                  # 512 free dim
NST = NT // ST_T                # 32 supertiles
NA = NT + NC                    # 320 columns in the packed activation tensor

# tensors that feed the PE as lhsT/rhs must be float32r
R_NAMES = {
    "w1k_n", "w1v_n", "w2k", "w2v", "w2v_n",
    "kctx_w", "vctx_w", "dctx_w", "wq_s", "ktgt_w", "vtgt_w", "dtgt_w",
    "wg1", "wg2", "wg3", "wkg1", "wvg2", "mask_qh", "e_hd", "ident",
}


def _r(ap):
    return ap


def _pack(a):
    """(256, M) -> (128, 2, M) with row d at [d % 128, d // 128, :]."""
    m = a.shape[1]
    return np.ascontiguousarray(a.reshape(2, 128, m).transpose(1, 0, 2))


def _packb(a):
    """(256,) -> (128, 2)."""
    return np.ascontiguousarray(a.reshape(2, 128).T)


def make_front(nc, w, sp, pp_h, pp_big, phiT, dups, gctx, bias_t,
               gbias, t0):
    """Issue dphi->h->K/V/D->gate->Kg/Vg for one supertile; returns state for
    the back half (scores/softmax/ctx)."""
    ndphiT = sp.tile([DPHI, C2], F32R, tag="ndphiT", name="ndphiT")
    for ti in range(ST_T):
        nc.vector.tensor_scalar_sub(
            ndphiT[:, ti * NC:(ti + 1) * NC], phiT[:, NT:NA],
            phiT[:, t0 + ti:t0 + ti + 1])

    hs = {}
    for nm in ("k", "v"):
        hps = pp_h.tile([128, C2], F32, tag="h", name="hps_" + nm)
        nc.tensor.matmul(hps[:], w["w1" + nm + "_n"][:], ndphiT[:],
                         start=True, stop=True)
        hs[nm] = sp.tile([128, C2], F32R, tag="h" + nm, name="hs_" + nm)
        nc.scalar.activation(hs[nm][:], hps[:], AF.Relu,
                             bias=w["b1" + nm][:])

    Kp = pp_big.tile([128, 2, C2], F32, tag="big", name="Kp")
    Vp = pp_big.tile([128, 2, C2], F32, tag="big", name="Vp")
    Dp = pp_big.tile([128, 2, C2], F32, tag="big", name="Dp")
    for mc in range(2):
        msl = slice(mc * 128, (mc + 1) * 128)
        nc.tensor.matmul(Kp[:, mc, :], w["w2k"][:, msl], hs["k"][:],
                         start=True, stop=False)
        nc.tensor.matmul(Kp[:, mc, :], w["ident"][:],
                         dups["kctxT"][:, mc, :], start=False, stop=True)
        nc.tensor.matmul(Vp[:, mc, :], w["w2v"][:, msl], hs["v"][:],
                         start=True, stop=False)
        nc.tensor.matmul(Vp[:, mc, :], w["ident"][:],
                         dups["vctxT"][:, mc, :], start=False, stop=True)
        nc.tensor.matmul(Dp[:, mc, :], w["w2k"][:, msl], hs["k"][:],
                         start=True, stop=False)
        nc.tensor.matmul(Dp[:, mc, :], w["w2v_n"][:, msl], hs["v"][:],
                         start=False, stop=False)
        nc.tensor.matmul(Dp[:, mc, :], w["ident"][:],
                         dups["dctxT"][:, mc, :], start=False, stop=True)

    dabs = sp.tile([128, 2, C2], F32R, tag="dabs", name="dabs")
    for mc in range(2):
        for ti in range(ST_T):
            csl = slice(ti * NC, (ti + 1) * NC)
            nc.scalar.activation(
                dabs[:, mc, csl], Dp[:, mc, csl], AF.Abs,
                bias=bias_t["bkv"][:, mc, t0 + ti:t0 + ti + 1].bitcast(F32))

    Gp = pp_big.tile([128, 2, C2], F32, tag="big", name="Gp")
    for mc in range(2):
        msl = slice(mc * 128, (mc + 1) * 128)
        nc.tensor.matmul(Gp[:, mc, :], w["wkg1"][:, msl], hs["k"][:],
                         start=True, stop=False)
        nc.tensor.matmul(Gp[:, mc, :], w["wvg2"][:, msl], hs["v"][:],
                         start=False, stop=False)
        for kc in range(2):
            nc.tensor.matmul(Gp[:, mc, :], w["wg3"][:, kc, msl],
                             dabs[:, kc, :], start=False, stop=False)
        nc.tensor.matmul(Gp[:, mc, :], w["ident"][:], gctx[:, mc, :],
                         start=False, stop=True)

    gs = sp.tile([128, 2, C2], F32, tag="gs", name="gs")
    for mc in range(2):
        for ti in range(ST_T):
            csl = slice(ti * NC, (ti + 1) * NC)
            nc.scalar.activation(
                gs[:, mc, csl], Gp[:, mc, csl], AF.Sigmoid,
                bias=gbias[:, mc, t0 + ti:t0 + ti + 1])

    Kg = sp.tile([128, 2, C2], F32R, tag="Kg", name="Kg")
    Vg = sp.tile([128, 2, C2], F32, tag="Vg", name="Vg")
    for mc in range(2):
        for ti in range(ST_T):
            csl = slice(ti * NC, (ti + 1) * NC)
            nc.vector.scalar_tensor_tensor(
                Kg[:, mc, csl], Kp[:, mc, csl],
                bias_t["bk"][:, mc, t0 + ti:t0 + ti + 1].bitcast(F32),
                gs[:, mc, csl], ALU.add, ALU.mult)
            nc.vector.scalar_tensor_tensor(
                Vg[:, mc, csl], Vp[:, mc, csl],
                bias_t["bv"][:, mc, t0 + ti:t0 + ti + 1].bitcast(F32),
                gs[:, mc, csl], ALU.add, ALU.mult)

    qb = sp.tile([128, 2, ST_T, H], F32R, tag="qb", name="qb")
    for ti in range(ST_T):
        for dc in range(2):
            nc.vector.tensor_scalar_mul(
                qb[:, dc, ti, :], w["mask_qh"][:, dc, :],
                bias_t["q"][:, dc, t0 + ti:t0 + ti + 1].bitcast(F32))
    return (Kg, Vg, qb, t0)


def run_back(nc, w, sp, pp_h, pp_big, ctx_all, state):
    Kg, Vg, qb, col0 = state
    Sps = pp_h.tile([128, C2], F32, tag="h", name="Sps")
    for ti in range(ST_T):
        csl = slice(ti * NC, (ti + 1) * NC)
        for dc in range(2):
            nc.tensor.matmul(Sps[0:H, csl], qb[:, dc, ti, :],
                             Kg[:, dc, csl], start=(dc == 0), stop=(dc == 1))

    attn_u = sp.tile([H, C2], F32, tag="attn_u", name="attn_u")
    rowsum = sp.tile([H, ST_T], F32, tag="rowsum", name="rowsum")
    for ti in range(ST_T):
        csl = slice(ti * NC, (ti + 1) * NC)
        nc.scalar.activation(attn_u[:, csl], Sps[0:H, csl], AF.Exp,
                             accum_out=rowsum[:, ti:ti + 1])
    rsr = sp.tile([H, ST_T], F32, tag="rsr", name="rsr")
    nc.vector.reciprocal(rsr[:], rowsum[:])
    attn_n = sp.tile([H, C2], F32R, tag="attn_n", name="attn_n")
    for ti in range(ST_T):
        csl = slice(ti * NC, (ti + 1) * NC)
        nc.vector.tensor_scalar_mul(attn_n[:, csl], attn_u[:, csl],
                                    rsr[:, ti:ti + 1])

    for dc in range(2):
        Ax = pp_h.tile([128, C2], F32, tag="h", name="Ax")
        nc.tensor.matmul(Ax[:], w["e_hd"][:, dc * 128:(dc + 1) * 128],
                         attn_n[:], start=True, stop=True)
        for ti in range(ST_T):
            csl = slice(ti * NC, (ti + 1) * NC)
            scr = sp.tile([128, NC], F32, tag="scr", name="scr")
            nc.vector.scalar_tensor_tensor(
                scr[:], Vg[:, dc, csl], 0.0, Ax[:, csl],
                ALU.add, ALU.mult,
                accum_out=ctx_all[:, dc, col0 + ti:col0 + ti + 1])


def build_kernel(wv):
    """wv: dict of packed numpy weight arrays; baked into the NEFF as Consts."""
    # disable_frame_to_traceback: recorded tracebacks embed the CALLER's
    # file/line into the BIR debug_table, which leaks into the jax
    # persistent-cache key and forces a recompile per calling script.
    nc = bacc.Bacc("TRN2", target_bir_lowering=False, debug=False,
                   disable_frame_to_traceback=True)

    dr_act = nc.dram_tensor("act", [128, 2, NA], BF16, kind="ExternalInput")
    dr_phi = nc.dram_tensor("phi", [DPHI, NA], F32, kind="ExternalInput")
    out_d = nc.dram_tensor("out_t", [128, 2, NT], BF16, kind="ExternalOutput")

    dr_w = {k: nc.inline_tensor(v, name="cw_" + k) for k, v in wv.items()}

    with ExitStack() as ctx:
        tc = ctx.enter_context(tile.TileContext(nc))
        wp = ctx.enter_context(tc.tile_pool(name="w", bufs=1))
        sp = ctx.enter_context(tc.tile_pool(name="sp", bufs=2))
        acc = ctx.enter_context(tc.tile_pool(name="acc", bufs=1))
        pp_h = ctx.enter_context(
            tc.tile_pool(name="pph", bufs=2, space="PSUM"))
        pp_big = ctx.enter_context(
            tc.tile_pool(name="ppb", bufs=3, space="PSUM"))

        # bf16 consts are DMA'd into bf16 staging tiles, then upcast into the
        # float32r tiles the PE consumes (alternating engines for overlap).
        w = {}
        upcast_i = 0
        for k, v in wv.items():
            if v.dtype == BF16NP:
                stg = wp.tile(list(v.shape), BF16, tag="s_" + k,
                              name="s_" + k)
                nc.sync.dma_start(out=stg[:], in_=dr_w[k].ap())
                w[k] = wp.tile(list(v.shape), F32R, tag=k, name="w_" + k)
                if upcast_i % 2 == 0:
                    nc.vector.tensor_copy(w[k][:], stg[:])
                else:
                    nc.scalar.activation(w[k][:], stg[:], AF.Identity)
                upcast_i += 1
            else:
                w[k] = wp.tile(list(v.shape), F32, tag=k, name="w_" + k)
                nc.sync.dma_start(out=w[k][:], in_=dr_w[k].ap())

        actS = wp.tile([128, 2, NA], BF16, tag="actS", name="actS")
        nc.sync.dma_start(out=actS[:], in_=dr_act.ap())
        actT = wp.tile([128, 2, NA], F32R, tag="actT", name="actT")
        nc.vector.tensor_copy(actT[:], actS[:])
        phiT = wp.tile([DPHI, NA], F32, tag="phiT", name="phiT")
        nc.sync.dma_start(out=phiT[:], in_=dr_phi.ap())

        def rtT(kc):
            return actT[:, kc, 0:NT]

        def rctxT(kc):
            return actT[:, kc, NT:NA]

        ctx_all = acc.tile([128, 2, NT], F32, tag="ctx_all")

        # ---- per-core precomputes (one b per core) ----
        # ctx projections, duplicated twice along free dim so a single
        # N=512 identity-matmul injects them into two-target PSUM tiles.
        dups = {}
        for nm, wt in (("kctxT", "kctx_w"), ("vctxT", "vctx_w"),
                       ("dctxT", "dctx_w")):
            dups[nm] = wp.tile([128, 2, C2], F32R, tag=nm, name="dup_" + nm)
            for mc in range(2):
                ps = pp_h.tile([128, C2], F32, tag="h")
                for kc in range(2):
                    nc.tensor.matmul(
                        ps[:, 0:NC],
                        _r(w[wt][:, kc, mc * 128:(mc + 1) * 128]),
                        _r(rctxT(kc)),
                        start=(kc == 0), stop=(kc == 1))
                for rep in range(2):
                    dst = dups[nm][:, mc, rep * NC:(rep + 1) * NC]
                    if mc == 0:
                        nc.scalar.activation(dst, ps[:, 0:NC], AF.Identity)
                    else:
                        nc.vector.tensor_copy(dst, ps[:, 0:NC])

        gctx = wp.tile([128, 2, C2], F32R, tag="gctx")
        for mc in range(2):
            ps = pp_h.tile([128, C2], F32, tag="h")
            i = 0
            for wt, src in (("wg1", "kctxT"), ("wg2", "vctxT")):
                for kc in range(2):
                    nc.tensor.matmul(
                        ps[:, 0:NC],
                        _r(w[wt][:, kc, mc * 128:(mc + 1) * 128]),
                        _r(dups[src][:, kc, 0:NC]),
                        start=(i == 0), stop=(i == 3))
                    i += 1
            for rep in range(2):
                dst = gctx[:, mc, rep * NC:(rep + 1) * NC]
                if mc == 0:
                    nc.scalar.activation(dst, ps[:, 0:NC], AF.Identity)
                else:
                    nc.vector.tensor_copy(dst, ps[:, 0:NC])

        # per-target bias vectors: bias_k = ktgt_w^T R_t^T + b2k, etc.
        bias_t = {}
        for nm, wt, bb in (("bk", "ktgt_w", "b2k"), ("bv", "vtgt_w", "b2v"),
                           ("bkv", "dtgt_w", "db2"), ("q", "wq_s", "bq_s")):
            bias_t[nm] = wp.tile([128, 2, NT], F32R, tag="bt_" + nm,
                                 name="bt_" + nm)
            for mc in range(2):
                ps = pp_h.tile([128, C2], F32, tag="h")
                for kc in range(2):
                    nc.tensor.matmul(
                        ps[:, 0:NT],
                        _r(w[wt][:, kc, mc * 128:(mc + 1) * 128]),
                        _r(rtT(kc)),
                        start=(kc == 0), stop=(kc == 1))
                nc.scalar.activation(
                    bias_t[nm][:, mc, :], ps[:, 0:NT], AF.Identity,
                    bias=w[bb][:, mc:mc + 1])

        # gate bias per target: wg1^T bias_k + wg2^T bias_v + gate_b
        gbias = wp.tile([128, 2, NT], F32, tag="gbias")
        for mc in range(2):
            ps = pp_h.tile([128, C2], F32, tag="h")
            i = 0
            for wt, src in (("wg1", "bk"), ("wg2", "bv")):
                for kc in range(2):
                    nc.tensor.matmul(
                        ps[:, 0:NT],
                        _r(w[wt][:, kc, mc * 128:(mc + 1) * 128]),
                        _r(bias_t[src][:, kc, :]),
                        start=(i == 0), stop=(i == 3))
                    i += 1
            nc.scalar.activation(
                gbias[:, mc, :], ps[:, 0:NT], AF.Identity,
                bias=w["gate_b"][:, mc:mc + 1])

        # ---- supertiles: 2 targets, free dim 512 ----
        # (front halves are queued; back halves are issued one iteration
        # later so each engine always has independent work in flight)
        pending = []

        def drain_one():
            if pending:
                run_back(nc, w, sp, pp_h, pp_big, ctx_all, pending.pop(0))

        for st in range(NST):
            t0 = st * ST_T
            st_state = make_front(nc, w, sp, pp_h, pp_big,
                                  phiT, dups, gctx, bias_t, gbias, t0)
            drain_one()
            pending.append(st_state)

        drain_one()

        # ---- output projection: out^T = out_w^T @ ctx_all + out_b ----
        outT = acc.tile([128, 2, NT], BF16, tag="outT")
        for mc in range(2):
            ps = pp_h.tile([128, C2], F32, tag="h")
            for kc in range(2):
                nc.tensor.matmul(
                    ps[:, 0:NT],
                    _r(w["out_w"][:, kc, mc * 128:(mc + 1) * 128]),
                    _r(ctx_all[:, kc, :]),
                    start=(kc == 0), stop=(kc == 1))
            nc.scalar.activation(outT[:, mc, :], ps[:, 0:NT], AF.Identity,
                                 bias=w["out_b"][:, mc:mc + 1])
        nc.sync.dma_start(out=out_d.ap(), in_=outT[:])

    nc.compile()

    # Normalize per-instruction debug info: recorded tracebacks / absolute
    # file paths otherwise leak the caller's script and kernel.py's location
    # into the serialized BIR, which would make the jax persistent-cache key
    # differ per caller and per checkout path (forcing a spurious recompile).
    def canon(d):
        return type(d)(
            op_name=d.op_name, tensorizer_id=d.tensorizer_id,
            filename="k.py", lineno=0,
            bass_funcname=d.bass_funcname, kernel_name=d.kernel_name,
            ant_traceback=None, ant_layer=d.ant_layer,
            ant_annotation=d.ant_annotation)

    for fn in nc.m.functions:
        for blk in fn.blocks:
            for inst in blk.instructions:
                if inst.debug is not None:
                    inst.debug = canon(inst.debug)
        for alloc in fn.allocations:
            mls = getattr(alloc, "memorylocations", None) or []
            for ml in mls:
                if getattr(ml, "ant_debug", None) is not None:
                    ml.ant_debug = canon(ml.ant_debug)
    return nc


_NC_CACHE = {}
_DISK_DIR = "/tmp/bass_kernel_cache"


class _NcShim:
    """Duck-typed stand-in for the Bass object on run_bass_kernel_spmd's axon
    path: exposes the compiled module plus the handful of attributes the
    bass2jax lowering reads, with to_json_bytes() returning the cached
    serialization (skips re-serializing the module on every call, and lets a
    fresh process skip the whole tile-framework build via the disk cache)."""

    target_bir_lowering = False
    partition_id_tensor = None
    dbg_addr = None
    debug = False
    dbg_callbacks = ()
    has_collectives = False

    class _PidT:
        name = "partition_id"

    def __init__(self, m, json_bytes):
        self.m = m
        self._json = json_bytes
        for alloc in m.functions[0].allocations:
            if (isinstance(alloc, mybir.MemoryLocationSet)
                    and alloc.kind == "ExternalInput"
                    and alloc.memorylocations
                    and alloc.memorylocations[0].name == "partition_id"):
                self.partition_id_tensor = self._PidT()
                break

    def to_json_bytes(self):
        return self._json


def _get_nc(key, inputs):
    if key in _NC_CACHE:
        return _NC_CACHE[key]
    path = os.path.join(_DISK_DIR, key + ".birj")
    shim = None
    if os.path.exists(path):
        try:
            j = open(path, "rb").read()
            shim = _NcShim(mybir.module_from_json_bytes(j), j)
        except Exception:
            shim = None
    if shim is None:
        nc = build_kernel(_marshal_weights(inputs))
        j = nc.to_json_bytes()
        shim = _NcShim(nc.m, j)
        try:
            os.makedirs(_DISK_DIR, exist_ok=True)
            tmp = f"{path}.tmp{os.getpid()}"
            with open(tmp, "wb") as f:
                f.write(j)
            os.replace(tmp, path)
        except Exception:
            pass
    _NC_CACHE[key] = shim
    return shim


_WEIGHT_KEYS = (
    "Wq_w", "Wq_b", "kctx_w", "ktgt_w", "kphi_w1", "kphi_b1", "kphi_w2",
    "kphi_b2", "vctx_w", "vtgt_w", "vphi_w1", "vphi_b1", "vphi_w2", "vphi_b2",
    "gate_w", "gate_b", "out_w", "out_b",
)


def _whash(inputs):
    h = hashlib.blake2b(digest_size=16)
    for k in _WEIGHT_KEYS:
        a = np.ascontiguousarray(np.asarray(inputs[k], np.float32))
        h.update(k.encode())
        h.update(str(a.shape).encode())
        h.update(a.tobytes())
    return h.hexdigest()


def _marshal_weights(inputs):
    f32 = np.float32
    gw = np.asarray(inputs["gate_w"], f32)
    wg1, wg2, wg3 = gw[0:256], gw[256:512], gw[512:768]
    kphi_w2 = np.asarray(inputs["kphi_w2"], f32)
    vphi_w2 = np.asarray(inputs["vphi_w2"], f32)
    sc = 1.0 / np.sqrt(DK)

    mask = np.zeros((256, H), f32)
    for d in range(256):
        mask[d, d // 32] = 1.0
    e_hd = np.ascontiguousarray(mask.T)
    mask_p = _pack(mask)

    wv = {
        "w1k_n": -np.asarray(inputs["kphi_w1"], f32),
        "w1v_n": -np.asarray(inputs["vphi_w1"], f32),
        "b1k": np.asarray(inputs["kphi_b1"], f32).reshape(HID, 1),
        "b1v": np.asarray(inputs["vphi_b1"], f32).reshape(HID, 1),
        "w2k": kphi_w2, "w2v": vphi_w2, "w2v_n": -vphi_w2,
        "kctx_w": _pack(np.asarray(inputs["kctx_w"], f32)),
        "vctx_w": _pack(np.asarray(inputs["vctx_w"], f32)),
        "dctx_w": _pack(np.asarray(inputs["kctx_w"], f32)
                        - np.asarray(inputs["vctx_w"], f32)),
        "wq_s": _pack(np.asarray(inputs["Wq_w"], f32) * sc),
        "bq_s": _packb(np.asarray(inputs["Wq_b"], f32) * sc),
        "ktgt_w": _pack(np.asarray(inputs["ktgt_w"], f32)),
        "vtgt_w": _pack(np.asarray(inputs["vtgt_w"], f32)),
        "dtgt_w": _pack(np.asarray(inputs["ktgt_w"], f32)
                        - np.asarray(inputs["vtgt_w"], f32)),
        "b2k": _packb(np.asarray(inputs["kphi_b2"], f32)),
        "b2v": _packb(np.asarray(inputs["vphi_b2"], f32)),
        "db2": _packb(np.asarray(inputs["kphi_b2"], f32)
                      - np.asarray(inputs["vphi_b2"], f32)),
        "wg1": _pack(wg1), "wg2": _pack(wg2), "wg3": _pack(wg3),
        "wkg1": np.ascontiguousarray(kphi_w2 @ wg1),
        "wvg2": np.ascontiguousarray(vphi_w2 @ wg2),
        "gate_b": _packb(np.asarray(inputs["gate_b"], f32)),
        "out_w": _pack(np.asarray(inputs["out_w"], f32)),
        "out_b": _packb(np.asarray(inputs["out_b"], f32)),
        "mask_qh": mask_p, "e_hd": e_hd, "ident": np.eye(128, dtype=f32),
    }
    return {k: np.ascontiguousarray(
                np.asarray(v, f32).astype(BF16NP) if k in R_NAMES
                else np.asarray(v, f32))
            for k, v in wv.items()}


def kernel(**inputs):
    f32 = np.float32
    key = _whash(inputs)
    nc = _get_nc(key, inputs)

    R_t = np.asarray(inputs["R_t"], f32)
    R_ctx = np.asarray(inputs["R_ctx"], f32)
    phi_t = np.asarray(inputs["phi_t"], f32)
    phi_c = np.asarray(inputs["phi_c"], f32)

    in_maps = []
    for core in range(NCORES):
        b, hh = core // 2, core % 2
        tsl = slice(hh * NT, (hh + 1) * NT)
        rt_p = R_t[b, tsl].T.reshape(2, 128, NT).transpose(1, 0, 2)
        rctx_p = R_ctx[b].T.reshape(2, 128, NC).transpose(1, 0, 2)
        act = np.ascontiguousarray(
            np.concatenate([rt_p, rctx_p], axis=2).astype(BF16NP))
        phi = np.ascontiguousarray(
            np.concatenate([phi_t[b, tsl].T, phi_c[b].T], axis=1))
        in_maps.append({"act": act, "phi": phi})

    res = run_bass_kernel_spmd(nc, in_maps, core_ids=list(range(NCORES)))
    kernel.last_results = res

    out = np.empty((B, NT_FULL, D), f32)
    for core in range(NCORES):
        r = res.results[core]["out_t"]            # (128, 2, 64) bf16
        arr = r.transpose(2, 1, 0).reshape(NT, D)
        b, hh = core // 2, core % 2
        out[b, hh * NT:(hh + 1) * NT, :] = arr.astype(f32)
    return out


# revision 31
# speedup vs baseline: 1.2819x; 1.2819x over previous
"""Bass/Trainium2 kernel for nn_HCTargetAwareAttnNP.

Sharding: 8 cores = B(4) x Nt-half(2). Each core handles one batch b and 64
of the 128 targets, with full R_ctx[b]/phi_c[b] local (softmax over Nc stays
on-core, no collectives).

Host-path design (the wall-clock bottleneck, not device FLOPs):
- All weights are baked into the NEFF as Const tensors (nc.inline_tensor),
  keyed by a hash of the weight inputs -- so per-call PJRT traffic is just
  two small activation tensors per core (~350 KB) instead of ~5 MB of
  replicated weights per core.
- The jax persistent compilation cache is enabled so the per-call
  jax.jit(shard_map(...)) inside run_bass_kernel_spmd deserializes the
  compiled executable instead of re-running XLA/neuronx-cc.

Device layout: everything FEATURE-MAJOR (feature dim on SBUF partitions,
context positions on the free dim); weight matrices are used in native
(in x out) layout as the PE stationary operand, and the pairwise (Nc x D)
tensors per (b,t) are built directly in PSUM by accumulating matmuls. Two
targets per supertile (free dim 512 = 2 x Nc).
"""

import hashlib
import os
import numpy as np
import ml_dtypes
from contextlib import ExitStack

BF16NP = np.float16

import jax

for _k, _v in (
    ("jax_compilation_cache_dir", "/tmp/bass_jax_pcc"),
    ("jax_persistent_cache_min_compile_time_secs", 0.0),
    ("jax_persistent_cache_min_entry_size_bytes", 0),
    # source locations otherwise leak the caller's filename/lineno into the
    # MLIR module, so the persistent-cache key would differ per caller script
    ("jax_include_full_tracebacks_in_locations", False),
    ("jax_traceback_in_locations_limit", 0),
):
    try:
        jax.config.update(_k, _v)
    except Exception:
        pass

import concourse.bass as bass
import concourse.tile as tile
from concourse import bacc, mybir
from concourse.bass_utils import run_bass_kernel_spmd

F32 = mybir.dt.float32
F32R = mybir.dt.float32r
BF16 = mybir.dt.float16
AF = mybir.ActivationFunctionType
ALU = mybir.AluOpType

B, NT_FULL, NC, D, DPHI, HID, H, DK = 4, 128, 256, 256, 16, 128, 8, 32
NCORES = 8
NT = 64                         # local targets per core (half of Nt)
ST_T = 2                        # targets per supertile
C2 = ST_T * NC                  # 512 free dim
NST = NT // ST_T                # 32 supertiles
NA = NT + NC                    # 320 columns in the packed activation tensor

# tensors that feed the PE as lhsT/rhs must be float32r
R_NAMES = {
    "w1k_n", "w1v_n", "w2k", "w2v", "w2v_n",
    "kctx_w", "vctx_w", "dctx_w", "wq_s", "ktgt_w", "vtgt_w", "dtgt_w",
    "wg1", "wg2", "wg3", "wkg1", "wvg2", "mask_qh", "e_hd", "ident",
}


def _r(ap):
    return ap


def _pack(a):
    """(256, M) -> (128, 2, M) with row d at [d % 128, d // 128, :]."""
    m = a.shape[1]
    return np.ascontiguousarray(a.reshape(2, 128, m).transpose(1, 0, 2))


def _packb(a):
    """(256,) -> (128, 2)."""
    return np.ascontiguousarray(a.reshape(2, 128).T)


def make_front(nc, w, sp, pp_h, pp_big, phiT, dups, gctx, bias_t,
               gbias, t0):
    """Issue dphi->h->K/V/D->gate->Kg/Vg for one supertile; returns state for
    the back half (scores/softmax/ctx)."""
    ndphiT = sp.tile([DPHI, C2], F32R, tag="ndphiT", name="ndphiT")
    for ti in range(ST_T):
        nc.vector.tensor_scalar_sub(
            ndphiT[:, ti * NC:(ti + 1) * NC], phiT[:, NT:NA],
            phiT[:, t0 + ti:t0 + ti + 1])

    hs = {}
    for nm in ("k", "v"):
        hps = pp_h.tile([128, C2], F32, tag="h", name="hps_" + nm)
        nc.tensor.matmul(hps[:], w["w1" + nm + "_n"][:], ndphiT[:],
                         start=True, stop=True)
        hs[nm] = sp.tile([128, C2], F32R, tag="h" + nm, name="hs_" + nm)
        nc.scalar.activation(hs[nm][:], hps[:], AF.Relu,
                             bias=w["b1" + nm][:])

    Kp = pp_big.tile([128, 2, C2], F32, tag="big", name="Kp")
    Vp = pp_big.tile([128, 2, C2], F32, tag="big", name="Vp")
    Dp = pp_big.tile([128, 2, C2], F32, tag="big", name="Dp")
    for mc in range(2):
        msl = slice(mc * 128, (mc + 1) * 128)
        nc.tensor.matmul(Kp[:, mc, :], w["w2k"][:, msl], hs["k"][:],
                         start=True, stop=False)
        nc.tensor.matmul(Kp[:, mc, :], w["ident"][:],
                         dups["kctxT"][:, mc, :], start=False, stop=True)
        nc.tensor.matmul(Vp[:, mc, :], w["w2v"][:, msl], hs["v"][:],
                         start=True, stop=False)
        nc.tensor.matmul(Vp[:, mc, :], w["ident"][:],
                         dups["vctxT"][:, mc, :], start=False, stop=True)
        nc.tensor.matmul(Dp[:, mc, :], w["w2k"][:, msl], hs["k"][:],
                         start=True, stop=False)
        nc.tensor.matmul(Dp[:, mc, :], w["w2v_n"][:, msl], hs["v"][:],
                         start=False, stop=False)
        nc.tensor.matmul(Dp[:, mc, :], w["ident"][:],
                         dups["dctxT"][:, mc, :], start=False, stop=True)

    dabs = sp.tile([128, 2, C2], F32R, tag="dabs", name="dabs")
    for mc in range(2):
        for ti in range(ST_T):
            csl = slice(ti * NC, (ti + 1) * NC)
            nc.scalar.activation(
                dabs[:, mc, csl], Dp[:, mc, csl], AF.Abs,
                bias=bias_t["bkv"][:, mc, t0 + ti:t0 + ti + 1].bitcast(F32))

    Gp = pp_big.tile([128, 2, C2], F32, tag="big", name="Gp")
    for mc in range(2):
        msl = slice(mc * 128, (mc + 1) * 128)
        nc.tensor.matmul(Gp[:, mc, :], w["wkg1"][:, msl], hs["k"][:],
                         start=True, stop=False)
        nc.tensor.matmul(Gp[:, mc, :], w["wvg2"][:, msl], hs["v"][:],
                         start=False, stop=False)
        for kc in range(2):
            nc.tensor.matmul(Gp[:, mc, :], w["wg3"][:, kc, msl],
                             dabs[:, kc, :], start=False, stop=False)
        nc.tensor.matmul(Gp[:, mc, :], w["ident"][:], gctx[:, mc, :],
                         start=False, stop=True)

    gs = sp.tile([128, 2, C2], F32, tag="gs", name="gs")
    for mc in range(2):
        for ti in range(ST_T):
            csl = slice(ti * NC, (ti + 1) * NC)
            nc.scalar.activation(
                gs[:, mc, csl], Gp[:, mc, csl], AF.Sigmoid,
                bias=gbias[:, mc, t0 + ti:t0 + ti + 1])

    Kg = sp.tile([128, 2, C2], F32R, tag="Kg", name="Kg")
    Vg = sp.tile([128, 2, C2], F32, tag="Vg", name="Vg")
    for mc in range(2):
        for ti in range(ST_T):
            csl = slice(ti * NC, (ti + 1) * NC)
            nc.vector.scalar_tensor_tensor(
                Kg[:, mc, csl], Kp[:, mc, csl],
                bias_t["bk"][:, mc, t0 + ti:t0 + ti + 1].bitcast(F32),
                gs[:, mc, csl], ALU.add, ALU.mult)
            nc.vector.scalar_tensor_tensor(
                Vg[:, mc, csl], Vp[:, mc, csl],
                bias_t["bv"][:, mc, t0 + ti:t0 + ti + 1].bitcast(F32),
                gs[:, mc, csl], ALU.add, ALU.mult)

    qb = sp.tile([128, 2, ST_T, H], F32R, tag="qb", name="qb")
    for ti in range(ST_T):
        for dc in range(2):
            nc.vector.tensor_scalar_mul(
                qb[:, dc, ti, :], w["mask_qh"][:, dc, :],
                bias_t["q"][:, dc, t0 + ti:t0 + ti + 1].bitcast(F32))
    return (Kg, Vg, qb, t0)


def run_back(nc, w, sp, pp_h, pp_big, ctx_all, state):
    Kg, Vg, qb, col0 = state
    Sps = pp_h.tile([128, C2], F32, tag="h", name="Sps")
    for ti in range(ST_T):
        csl = slice(ti * NC, (ti + 1) * NC)
        for dc in range(2):
            nc.tensor.matmul(Sps[0:H, csl], qb[:, dc, ti, :],
                             Kg[:, dc, csl], start=(dc == 0), stop=(dc == 1))

    attn_u = sp.tile([H, C2], F32, tag="attn_u", name="attn_u")
    rowsum = sp.tile([H, ST_T], F32, tag="rowsum", name="rowsum")
    for ti in range(ST_T):
        csl = slice(ti * NC, (ti + 1) * NC)
        nc.scalar.activation(attn_u[:, csl], Sps[0:H, csl], AF.Exp,
                             accum_out=rowsum[:, ti:ti + 1])
    rsr = sp.tile([H, ST_T], F32, tag="rsr", name="rsr")
    nc.vector.reciprocal(rsr[:], rowsum[:])
    attn_n = sp.tile([H, C2], F32R, tag="attn_n", name="attn_n")
    for ti in range(ST_T):
        csl = slice(ti * NC, (ti + 1) * NC)
        nc.vector.tensor_scalar_mul(attn_n[:, csl], attn_u[:, csl],
                                    rsr[:, ti:ti + 1])

    for dc in range(2):
        Ax = pp_h.tile([128, C2], F32, tag="h", name="Ax")
        nc.tensor.matmul(Ax[:], w["e_hd"][:, dc * 128:(dc + 1) * 128],
                         attn_n[:], start=True, stop=True)
        for ti in range(ST_T):
            csl = slice(ti * NC, (ti + 1) * NC)
            scr = sp.tile([128, NC], F32, tag="scr", name="scr")
            nc.vector.scalar_tensor_tensor(
                scr[:], Vg[:, dc, csl], 0.0, Ax[:, csl],
                ALU.add, ALU.mult,
                accum_out=ctx_all[:, dc, col0 + ti:col0 + ti + 1])


def build_kernel(wv):
    """wv: dict of packed numpy weight arrays; baked into the NEFF as Consts."""
    # disable_frame_to_traceback: recorded tracebacks embed the CALLER's
    # file/line into the BIR debug_table, which leaks into the jax
    # persistent-cache key and forces a recompile per calling script.
    nc = bacc.Bacc("TRN2", target_bir_lowering=False, debug=False,
                   disable_frame_to_traceback=True)

    dr_act = nc.dram_tensor("act", [128, 2, NA], BF16, kind="ExternalInput")
    dr_phi = nc.dram_tensor("phi", [DPHI, NA], F32, kind="ExternalInput")
    out_d = nc.dram_tensor("out_t", [128, 2, NT], BF16, kind="ExternalOutput")

    dr_w = {k: nc.inline_tensor(v, name="cw_" + k) for k, v in wv.items()}

    with ExitStack() as ctx:
        tc = ctx.enter_context(tile.TileContext(nc))
        wp = ctx.enter_context(tc.tile_pool(name="w", bufs=1))
        sp = ctx.enter_context(tc.tile_pool(name="sp", bufs=2))
        acc = ctx.enter_context(tc.tile_pool(name="acc", bufs=1))
        pp_h = ctx.enter_context(
            tc.tile_pool(name="pph", bufs=2, space="PSUM"))
        pp_big = ctx.enter_context(
            tc.tile_pool(name="ppb", bufs=3, space="PSUM"))

        # bf16 consts are DMA'd into bf16 staging tiles, then upcast into the
        # float32r tiles the PE consumes (alternating engines for overlap).
        w = {}
        upcast_i = 0
        for k, v in wv.items():
            if v.dtype == BF16NP:
                stg = wp.tile(list(v.shape), BF16, tag="s_" + k,
                              name="s_" + k)
                nc.sync.dma_start(out=stg[:], in_=dr_w[k].ap())
                w[k] = wp.tile(list(v.shape), F32R, tag=k, name="w_" + k)
                if upcast_i % 2 == 0:
                    nc.vector.tensor_copy(w[k][:], stg[:])
                else:
                    nc.scalar.activation(w[k][:], stg[:], AF.Identity)
                upcast_i += 1
            else:
                w[k] = wp.tile(list(v.shape), F32, tag=k, name="w_" + k)
                nc.sync.dma_start(out=w[k][:], in_=dr_w[k].ap())

        actS = wp.tile([128, 2, NA], BF16, tag="actS", name="actS")
        nc.sync.dma_start(out=actS[:], in_=dr_act.ap())
        actT = wp.tile([128, 2, NA], F32R, tag="actT", name="actT")
        nc.vector.tensor_copy(actT[:], actS[:])
        phiT = wp.tile([DPHI, NA], F32, tag="phiT", name="phiT")
        nc.sync.dma_start(out=phiT[:], in_=dr_phi.ap())

        def rtT(kc):
            return actT[:, kc, 0:NT]

        def rctxT(kc):
            return actT[:, kc, NT:NA]

        ctx_all = acc.tile([128, 2, NT], F32, tag="ctx_all")

        # ---- per-core precomputes (one b per core) ----
        # ctx projections, duplicated twice along free dim so a single
        # N=512 identity-matmul injects them into two-target PSUM tiles.
        dups = {}
        for nm, wt in (("kctxT", "kctx_w"), ("vctxT", "vctx_w"),
                       ("dctxT", "dctx_w")):
            dups[nm] = wp.tile([128, 2, C2], F32R, tag=nm, name="dup_" + nm)
            for mc in range(2):
                ps = pp_h.tile([128, C2], F32, tag="h")
                for kc in range(2):
                    nc.tensor.matmul(
                        ps[:, 0:NC],
                        _r(w[wt][:, kc, mc * 128:(mc + 1) * 128]),
                        _r(rctxT(kc)),
                        start=(kc == 0), stop=(kc == 1))
                for rep in range(2):
                    dst = dups[nm][:, mc, rep * NC:(rep + 1) * NC]
                    if mc == 0:
                        nc.scalar.activation(dst, ps[:, 0:NC], AF.Identity)
                    else:
                        nc.vector.tensor_copy(dst, ps[:, 0:NC])

        gctx = wp.tile([128, 2, C2], F32R, tag="gctx")
        for mc in range(2):
            ps = pp_h.tile([128, C2], F32, tag="h")
            i = 0
            for wt, src in (("wg1", "kctxT"), ("wg2", "vctxT")):
                for kc in range(2):
                    nc.tensor.matmul(
                        ps[:, 0:NC],
                        _r(w[wt][:, kc, mc * 128:(mc + 1) * 128]),
                        _r(dups[src][:, kc, 0:NC]),
                        start=(i == 0), stop=(i == 3))
                    i += 1
            for rep in range(2):
                dst = gctx[:, mc, rep * NC:(rep + 1) * NC]
                if mc == 0:
                    nc.scalar.activation(dst, ps[:, 0:NC], AF.Identity)
                else:
                    nc.vector.tensor_copy(dst, ps[:, 0:NC])

        # per-target bias vectors: bias_k = ktgt_w^T R_t^T + b2k, etc.
        bias_t = {}
        for nm, wt, bb in (("bk", "ktgt_w", "b2k"), ("bv", "vtgt_w", "b2v"),
                           ("bkv", "dtgt_w", "db2"), ("q", "wq_s", "bq_s")):
            bias_t[nm] = wp.tile([128, 2, NT], F32R, tag="bt_" + nm,
                                 name="bt_" + nm)
            for mc in range(2):
                ps = pp_h.tile([128, C2], F32, tag="h")
                for kc in range(2):
                    nc.tensor.matmul(
                        ps[:, 0:NT],
                        _r(w[wt][:, kc, mc * 128:(mc + 1) * 128]),
                        _r(rtT(kc)),
                        start=(kc == 0), stop=(kc == 1))
                nc.scalar.activation(
                    bias_t[nm][:, mc, :], ps[:, 0:NT], AF.Identity,
                    bias=w[bb][:, mc:mc + 1])

        # gate bias per target: wg1^T bias_k + wg2^T bias_v + gate_b
        gbias = wp.tile([128, 2, NT], F32, tag="gbias")
        for mc in range(2):
            ps = pp_h.tile([128, C2], F32, tag="h")
            i = 0
            for wt, src in (("wg1", "bk"), ("wg2", "bv")):
                for kc in range(2):
                    nc.tensor.matmul(
                        ps[:, 0:NT],
                        _r(w[wt][:, kc, mc * 128:(mc + 1) * 128]),
                        _r(bias_t[src][:, kc, :]),
                        start=(i == 0), stop=(i == 3))
                    i += 1
            nc.scalar.activation(
                gbias[:, mc, :], ps[:, 0:NT], AF.Identity,
                bias=w["gate_b"][:, mc:mc + 1])

        # ---- supertiles: 2 targets, free dim 512 ----
        # (front halves are queued; back halves are issued one iteration
        # later so each engine always has independent work in flight)
        pending = []

        def drain_one():
            if pending:
                run_back(nc, w, sp, pp_h, pp_big, ctx_all, pending.pop(0))

        for st in range(NST):
            t0 = st * ST_T
            st_state = make_front(nc, w, sp, pp_h, pp_big,
                                  phiT, dups, gctx, bias_t, gbias, t0)
            drain_one()
            pending.append(st_state)

        drain_one()

        # ---- output projection: out^T = out_w^T @ ctx_all + out_b ----
        outT = acc.tile([128, 2, NT], BF16, tag="outT")
        for mc in range(2):
            ps = pp_h.tile([128, C2], F32, tag="h")
            for kc in range(2):
                nc.tensor.matmul(
                    ps[:, 0:NT],
                    _r(w["out_w"][:, kc, mc * 128:(mc + 1) * 128]),
                    _r(ctx_all[:, kc, :]),
                    start=(kc == 0), stop=(kc == 1))
            nc.scalar.activation(outT[:, mc, :], ps[:, 0:NT], AF.Identity,
                                 bias=w["out_b"][:, mc:mc + 1])
        nc.sync.dma_start(out=out_d.ap(), in_=outT[:])

    nc.compile()

    # Normalize per-instruction debug info: recorded tracebacks / absolute
    # file paths otherwise leak the caller's script and kernel.py's location
    # into the serialized BIR, which would make the jax persistent-cache key
    # differ per caller and per checkout path (forcing a spurious recompile).
    def canon(d):
        return type(d)(
            op_name=d.op_name, tensorizer_id=d.tensorizer_id,
            filename="k.py", lineno=0,
            bass_funcname=d.bass_funcname, kernel_name=d.kernel_name,
            ant_traceback=None, ant_layer=d.ant_layer,
            ant_annotation=d.ant_annotation)

    for fn in nc.m.functions:
        for blk in fn.blocks:
            for inst in blk.instructions:
                if inst.debug is not None:
                    inst.debug = canon(inst.debug)
        for alloc in fn.allocations:
            mls = getattr(alloc, "memorylocations", None) or []
            for ml in mls:
                if getattr(ml, "ant_debug", None) is not None:
                    ml.ant_debug = canon(ml.ant_debug)
    return nc


_NC_CACHE = {}
_DISK_DIR = "/tmp/bass_kernel_cache"


class _NcShim:
    """Duck-typed stand-in for the Bass object on run_bass_kernel_spmd's axon
    path: exposes the compiled module plus the handful of attributes the
    bass2jax lowering reads, with to_json_bytes() returning the cached
    serialization (skips re-serializing the module on every call, and lets a
    fresh process skip the whole tile-framework build via the disk cache)."""

    target_bir_lowering = False
    partition_id_tensor = None
    dbg_addr = None
    debug = False
    dbg_callbacks = ()
    has_collectives = False

    class _PidT:
        name = "partition_id"

    def __init__(self, m, json_bytes):
        self.m = m
        self._json = json_bytes
        for alloc in m.functions[0].allocations:
            if (isinstance(alloc, mybir.MemoryLocationSet)
                    and alloc.kind == "ExternalInput"
                    and alloc.memorylocations
                    and alloc.memorylocations[0].name == "partition_id"):
                self.partition_id_tensor = self._PidT()
                break

    def to_json_bytes(self):
        return self._json


def _get_nc(key, inputs):
    if key in _NC_CACHE:
        return _NC_CACHE[key]
    path = os.path.join(_DISK_DIR, key + ".birj")
    shim = None
    if os.path.exists(path):
        try:
            j = open(path, "rb").read()
            shim = _NcShim(mybir.module_from_json_bytes(j), j)
        except Exception:
            shim = None
    if shim is None:
        nc = build_kernel(_marshal_weights(inputs))
        j = nc.to_json_bytes()
        shim = _NcShim(nc.m, j)
        try:
            os.makedirs(_DISK_DIR, exist_ok=True)
            tmp = f"{path}.tmp{os.getpid()}"
            with open(tmp, "wb") as f:
                f.write(j)
            os.replace(tmp, path)
        except Exception:
            pass
    _NC_CACHE[key] = shim
    return shim


_WEIGHT_KEYS = (
    "Wq_w", "Wq_b", "kctx_w", "ktgt_w", "kphi_w1", "kphi_b1", "kphi_w2",
    "kphi_b2", "vctx_w", "vtgt_w", "vphi_w1", "vphi_b1", "vphi_w2", "vphi_b2",
    "gate_w", "gate_b", "out_w", "out_b",
)


_KVER = b"hc-attn-v4"  # bump when build_kernel's emitted program changes


def _whash(inputs):
    h = hashlib.blake2b(digest_size=16)
    h.update(_KVER)
    for k in _WEIGHT_KEYS:
        a = np.ascontiguousarray(np.asarray(inputs[k], np.float32))
        h.update(k.encode())
        h.update(str(a.shape).encode())
        h.update(a.tobytes())
    return h.hexdigest()


def _marshal_weights(inputs):
    f32 = np.float32
    gw = np.asarray(inputs["gate_w"], f32)
    wg1, wg2, wg3 = gw[0:256], gw[256:512], gw[512:768]
    kphi_w2 = np.asarray(inputs["kphi_w2"], f32)
    vphi_w2 = np.asarray(inputs["vphi_w2"], f32)
    sc = 1.0 / np.sqrt(DK)

    mask = np.zeros((256, H), f32)
    for d in range(256):
        mask[d, d // 32] = 1.0
    e_hd = np.ascontiguousarray(mask.T)
    mask_p = _pack(mask)

    wv = {
        "w1k_n": -np.asarray(inputs["kphi_w1"], f32),
        "w1v_n": -np.asarray(inputs["vphi_w1"], f32),
        "b1k": np.asarray(inputs["kphi_b1"], f32).reshape(HID, 1),
        "b1v": np.asarray(inputs["vphi_b1"], f32).reshape(HID, 1),
        "w2k": kphi_w2, "w2v": vphi_w2, "w2v_n": -vphi_w2,
        "kctx_w": _pack(np.asarray(inputs["kctx_w"], f32)),
        "vctx_w": _pack(np.asarray(inputs["vctx_w"], f32)),
        "dctx_w": _pack(np.asarray(inputs["kctx_w"], f32)
                        - np.asarray(inputs["vctx_w"], f32)),
        "wq_s": _pack(np.asarray(inputs["Wq_w"], f32) * sc),
        "bq_s": _packb(np.asarray(inputs["Wq_b"], f32) * sc),
        "ktgt_w": _pack(np.asarray(inputs["ktgt_w"], f32)),
        "vtgt_w": _pack(np.asarray(inputs["vtgt_w"], f32)),
        "dtgt_w": _pack(np.asarray(inputs["ktgt_w"], f32)
                        - np.asarray(inputs["vtgt_w"], f32)),
        "b2k": _packb(np.asarray(inputs["kphi_b2"], f32)),
        "b2v": _packb(np.asarray(inputs["vphi_b2"], f32)),
        "db2": _packb(np.asarray(inputs["kphi_b2"], f32)
                      - np.asarray(inputs["vphi_b2"], f32)),
        "wg1": _pack(wg1), "wg2": _pack(wg2), "wg3": _pack(wg3),
        "wkg1": np.ascontiguousarray(kphi_w2 @ wg1),
        "wvg2": np.ascontiguousarray(vphi_w2 @ wg2),
        "gate_b": _packb(np.asarray(inputs["gate_b"], f32)),
        "out_w": _pack(np.asarray(inputs["out_w"], f32)),
        "out_b": _packb(np.asarray(inputs["out_b"], f32)),
        "mask_qh": mask_p, "e_hd": e_hd, "ident": np.eye(128, dtype=f32),
    }
    return {k: np.ascontiguousarray(
                np.asarray(v, f32).astype(BF16NP) if k in R_NAMES
                else np.asarray(v, f32))
            for k, v in wv.items()}


def kernel(**inputs):
    f32 = np.float32
    key = _whash(inputs)
    nc = _get_nc(key, inputs)

    R_t = np.asarray(inputs["R_t"], f32)
    R_ctx = np.asarray(inputs["R_ctx"], f32)
    phi_t = np.asarray(inputs["phi_t"], f32)
    phi_c = np.asarray(inputs["phi_c"], f32)

    in_maps = []
    for core in range(NCORES):
        b, hh = core // 2, core % 2
        tsl = slice(hh * NT, (hh + 1) * NT)
        rt_p = R_t[b, tsl].T.reshape(2, 128, NT).transpose(1, 0, 2)
        rctx_p = R_ctx[b].T.reshape(2, 128, NC).transpose(1, 0, 2)
        act = np.ascontiguousarray(
            np.concatenate([rt_p, rctx_p], axis=2).astype(BF16NP))
        phi = np.ascontiguousarray(
            np.concatenate([phi_t[b, tsl].T, phi_c[b].T], axis=1))
        in_maps.append({"act": act, "phi": phi})

    res = run_bass_kernel_spmd(nc, in_maps, core_ids=list(range(NCORES)))
    kernel.last_results = res

    out = np.empty((B, NT_FULL, D), f32)
    for core in range(NCORES):
        r = res.results[core]["out_t"]            # (128, 2, 64) bf16
        arr = r.transpose(2, 1, 0).reshape(NT, D)
        b, hh = core // 2, core % 2
        out[b, hh * NT:(hh + 1) * NT, :] = arr.astype(f32)
    return out


# revision 32
# speedup vs baseline: 1.3836x; 1.0794x over previous
"""Bass/Trainium2 kernel for nn_HCTargetAwareAttnNP.

Sharding: 8 cores = B(4) x Nt-half(2). Each core handles one batch b and 64
of the 128 targets, with full R_ctx[b]/phi_c[b] local (softmax over Nc stays
on-core, no collectives).

Host-path design (the wall-clock bottleneck, not device FLOPs):
- All weights are baked into the NEFF as Const tensors (nc.inline_tensor),
  keyed by a hash of the weight inputs -- so per-call PJRT traffic is just
  two small activation tensors per core (~350 KB) instead of ~5 MB of
  replicated weights per core.
- The jax persistent compilation cache is enabled so the per-call
  jax.jit(shard_map(...)) inside run_bass_kernel_spmd deserializes the
  compiled executable instead of re-running XLA/neuronx-cc.

Device layout: everything FEATURE-MAJOR (feature dim on SBUF partitions,
context positions on the free dim); weight matrices are used in native
(in x out) layout as the PE stationary operand, and the pairwise (Nc x D)
tensors per (b,t) are built directly in PSUM by accumulating matmuls. Two
targets per supertile (free dim 512 = 2 x Nc).
"""

import hashlib
import os
import numpy as np
import ml_dtypes
from contextlib import ExitStack

BF16NP = np.float16

import jax

for _k, _v in (
    ("jax_compilation_cache_dir", "/tmp/bass_jax_pcc"),
    ("jax_persistent_cache_min_compile_time_secs", 0.0),
    ("jax_persistent_cache_min_entry_size_bytes", 0),
    # source locations otherwise leak the caller's filename/lineno into the
    # MLIR module, so the persistent-cache key would differ per caller script
    ("jax_include_full_tracebacks_in_locations", False),
    ("jax_traceback_in_locations_limit", 0),
):
    try:
        jax.config.update(_k, _v)
    except Exception:
        pass

import concourse.bass as bass
import concourse.tile as tile
from concourse import bacc, mybir
from concourse.bass_utils import run_bass_kernel_spmd

F32 = mybir.dt.float32
F32R = mybir.dt.float32r
BF16 = mybir.dt.float16
AF = mybir.ActivationFunctionType
ALU = mybir.AluOpType

B, NT_FULL, NC, D, DPHI, HID, H, DK = 4, 128, 256, 256, 16, 128, 8, 32
NCORES = 8
NT = 64                         # local targets per core (half of Nt)
ST_T = 2                        # targets per supertile
C2 = ST_T * NC                  # 512 free dim
NST = NT // ST_T                # 32 supertiles
NA = NT + NC                    # 320 columns in the packed activation tensor

# tensors that feed the PE as lhsT/rhs must be float32r
R_NAMES = {
    "w1k_n", "w1v_n", "w2k", "w2v", "w2v_n",
    "kctx_w", "vctx_w", "dctx_w", "wq_s", "ktgt_w", "vtgt_w", "dtgt_w",
    "wg1", "wg2", "wg3", "wkg1", "wvg2", "mask_qh", "e_hd", "ident",
}


def _r(ap):
    return ap


def _pack(a):
    """(256, M) -> (128, 2, M) with row d at [d % 128, d // 128, :]."""
    m = a.shape[1]
    return np.ascontiguousarray(a.reshape(2, 128, m).transpose(1, 0, 2))


def _packb(a):
    """(256,) -> (128, 2)."""
    return np.ascontiguousarray(a.reshape(2, 128).T)


def make_front(nc, w, sp, pp_h, pp_big, phiT, dups, gctx, bias_t,
               gbias, t0):
    """Issue dphi->h->K/V/D->gate->Kg/Vg for one supertile; returns state for
    the back half (scores/softmax/ctx)."""
    ndphiT = sp.tile([DPHI, C2], F32R, tag="ndphiT", name="ndphiT")
    for ti in range(ST_T):
        nc.vector.tensor_scalar_sub(
            ndphiT[:, ti * NC:(ti + 1) * NC], phiT[:, NT:NA],
            phiT[:, t0 + ti:t0 + ti + 1])

    hs = {}
    for nm in ("k", "v"):
        hps = pp_h.tile([128, C2], F32, tag="h", name="hps_" + nm)
        nc.tensor.matmul(hps[:], w["w1" + nm + "_n"][:], ndphiT[:],
                         start=True, stop=True)
        hs[nm] = sp.tile([128, C2], F32R, tag="h" + nm, name="hs_" + nm)
        nc.scalar.activation(hs[nm][:], hps[:], AF.Relu,
                             bias=w["b1" + nm][:])

    Kp = pp_big.tile([128, 2, C2], F32, tag="big", name="Kp")
    Vp = pp_big.tile([128, 2, C2], F32, tag="big", name="Vp")
    Dp = pp_big.tile([128, 2, C2], F32, tag="big", name="Dp")
    for mc in range(2):
        msl = slice(mc * 128, (mc + 1) * 128)
        nc.tensor.matmul(Kp[:, mc, :], w["w2k"][:, msl], hs["k"][:],
                         start=True, stop=False)
        nc.tensor.matmul(Kp[:, mc, :], w["ident"][:],
                         dups["kctxT"][:, mc, :], start=False, stop=True)
        nc.tensor.matmul(Vp[:, mc, :], w["w2v"][:, msl], hs["v"][:],
                         start=True, stop=False)
        nc.tensor.matmul(Vp[:, mc, :], w["ident"][:],
                         dups["vctxT"][:, mc, :], start=False, stop=True)
        nc.tensor.matmul(Dp[:, mc, :], w["w2k"][:, msl], hs["k"][:],
                         start=True, stop=False)
        nc.tensor.matmul(Dp[:, mc, :], w["w2v_n"][:, msl], hs["v"][:],
                         start=False, stop=False)
        nc.tensor.matmul(Dp[:, mc, :], w["ident"][:],
                         dups["dctxT"][:, mc, :], start=False, stop=True)

    dabs = sp.tile([128, 2, C2], F32R, tag="dabs", name="dabs")
    for mc in range(2):
        for ti in range(ST_T):
            csl = slice(ti * NC, (ti + 1) * NC)
            nc.scalar.activation(
                dabs[:, mc, csl], Dp[:, mc, csl], AF.Abs,
                bias=bias_t["bkv"][:, mc, t0 + ti:t0 + ti + 1].bitcast(F32))

    Gp = pp_big.tile([128, 2, C2], F32, tag="big", name="Gp")
    for mc in range(2):
        msl = slice(mc * 128, (mc + 1) * 128)
        nc.tensor.matmul(Gp[:, mc, :], w["wkg1"][:, msl], hs["k"][:],
                         start=True, stop=False)
        nc.tensor.matmul(Gp[:, mc, :], w["wvg2"][:, msl], hs["v"][:],
                         start=False, stop=False)
        for kc in range(2):
            nc.tensor.matmul(Gp[:, mc, :], w["wg3"][:, kc, msl],
                             dabs[:, kc, :], start=False, stop=False)
        nc.tensor.matmul(Gp[:, mc, :], w["ident"][:], gctx[:, mc, :],
                         start=False, stop=True)

    gs = sp.tile([128, 2, C2], F32, tag="gs", name="gs")
    for mc in range(2):
        for ti in range(ST_T):
            csl = slice(ti * NC, (ti + 1) * NC)
            nc.scalar.activation(
                gs[:, mc, csl], Gp[:, mc, csl], AF.Sigmoid,
                bias=gbias[:, mc, t0 + ti:t0 + ti + 1])

    Kg = sp.tile([128, 2, C2], F32R, tag="Kg", name="Kg")
    Vg = sp.tile([128, 2, C2], F32, tag="Vg", name="Vg")
    for mc in range(2):
        for ti in range(ST_T):
            csl = slice(ti * NC, (ti + 1) * NC)
            nc.vector.scalar_tensor_tensor(
                Kg[:, mc, csl], Kp[:, mc, csl],
                bias_t["bk"][:, mc, t0 + ti:t0 + ti + 1].bitcast(F32),
                gs[:, mc, csl], ALU.add, ALU.mult)
            nc.vector.scalar_tensor_tensor(
                Vg[:, mc, csl], Vp[:, mc, csl],
                bias_t["bv"][:, mc, t0 + ti:t0 + ti + 1].bitcast(F32),
                gs[:, mc, csl], ALU.add, ALU.mult)

    qb = sp.tile([128, 2, ST_T, H], F32R, tag="qb", name="qb")
    for ti in range(ST_T):
        for dc in range(2):
            nc.vector.tensor_scalar_mul(
                qb[:, dc, ti, :], w["mask_qh"][:, dc, :],
                bias_t["q"][:, dc, t0 + ti:t0 + ti + 1].bitcast(F32))
    return (Kg, Vg, qb, t0)


def run_back(nc, w, sp, pp_h, pp_big, ctx_all, state):
    Kg, Vg, qb, col0 = state
    Sps = pp_h.tile([128, C2], F32, tag="h", name="Sps")
    for ti in range(ST_T):
        csl = slice(ti * NC, (ti + 1) * NC)
        for dc in range(2):
            nc.tensor.matmul(Sps[0:H, csl], qb[:, dc, ti, :],
                             Kg[:, dc, csl], start=(dc == 0), stop=(dc == 1))

    attn_u = sp.tile([H, C2], F32, tag="attn_u", name="attn_u")
    rowsum = sp.tile([H, ST_T], F32, tag="rowsum", name="rowsum")
    for ti in range(ST_T):
        csl = slice(ti * NC, (ti + 1) * NC)
        nc.scalar.activation(attn_u[:, csl], Sps[0:H, csl], AF.Exp,
                             accum_out=rowsum[:, ti:ti + 1])
    rsr = sp.tile([H, ST_T], F32, tag="rsr", name="rsr")
    nc.vector.reciprocal(rsr[:], rowsum[:])
    attn_n = sp.tile([H, C2], F32R, tag="attn_n", name="attn_n")
    for ti in range(ST_T):
        csl = slice(ti * NC, (ti + 1) * NC)
        nc.vector.tensor_scalar_mul(attn_n[:, csl], attn_u[:, csl],
                                    rsr[:, ti:ti + 1])

    for dc in range(2):
        Ax = pp_h.tile([128, C2], F32, tag="h", name="Ax")
        nc.tensor.matmul(Ax[:], w["e_hd"][:, dc * 128:(dc + 1) * 128],
                         attn_n[:], start=True, stop=True)
        for ti in range(ST_T):
            csl = slice(ti * NC, (ti + 1) * NC)
            scr = sp.tile([128, NC], F32, tag="scr", name="scr")
            nc.vector.scalar_tensor_tensor(
                scr[:], Vg[:, dc, csl], 0.0, Ax[:, csl],
                ALU.add, ALU.mult,
                accum_out=ctx_all[:, dc, col0 + ti:col0 + ti + 1])


def build_kernel(wv):
    """wv: dict of packed numpy weight arrays; baked into the NEFF as Consts."""
    # disable_frame_to_traceback: recorded tracebacks embed the CALLER's
    # file/line into the BIR debug_table, which leaks into the jax
    # persistent-cache key and forces a recompile per calling script.
    nc = bacc.Bacc("TRN2", target_bir_lowering=False, debug=False,
                   disable_frame_to_traceback=True)

    dr_act = nc.dram_tensor("act", [128, 2, NA], BF16, kind="ExternalInput")
    dr_phi = nc.dram_tensor("phi", [DPHI, NA], F32, kind="ExternalInput")
    out_d = nc.dram_tensor("out_t", [128, 2, NT], BF16, kind="ExternalOutput")

    dr_w = {k: nc.inline_tensor(v, name="cw_" + k) for k, v in wv.items()}

    with ExitStack() as ctx:
        tc = ctx.enter_context(tile.TileContext(nc))
        wp = ctx.enter_context(tc.tile_pool(name="w", bufs=1))
        sp = ctx.enter_context(tc.tile_pool(name="sp", bufs=2))
        acc = ctx.enter_context(tc.tile_pool(name="acc", bufs=1))
        pp_h = ctx.enter_context(
            tc.tile_pool(name="pph", bufs=2, space="PSUM"))
        pp_big = ctx.enter_context(
            tc.tile_pool(name="ppb", bufs=3, space="PSUM"))

        # bf16 consts are DMA'd into bf16 staging tiles, then upcast into the
        # float32r tiles the PE consumes (alternating engines for overlap).
        w = {}
        upcast_i = 0
        for k, v in wv.items():
            if v.dtype == BF16NP:
                stg = wp.tile(list(v.shape), BF16, tag="s_" + k,
                              name="s_" + k)
                nc.sync.dma_start(out=stg[:], in_=dr_w[k].ap())
                w[k] = wp.tile(list(v.shape), F32R, tag=k, name="w_" + k)
                if upcast_i % 2 == 0:
                    nc.vector.tensor_copy(w[k][:], stg[:])
                else:
                    nc.scalar.activation(w[k][:], stg[:], AF.Identity)
                upcast_i += 1
            else:
                w[k] = wp.tile(list(v.shape), F32, tag=k, name="w_" + k)
                nc.sync.dma_start(out=w[k][:], in_=dr_w[k].ap())

        actS = wp.tile([128, 2, NA], BF16, tag="actS", name="actS")
        nc.sync.dma_start(out=actS[:], in_=dr_act.ap())
        actT = wp.tile([128, 2, NA], F32R, tag="actT", name="actT")
        nc.vector.tensor_copy(actT[:], actS[:])
        phiT = wp.tile([DPHI, NA], F32, tag="phiT", name="phiT")
        nc.sync.dma_start(out=phiT[:], in_=dr_phi.ap())

        def rtT(kc):
            return actT[:, kc, 0:NT]

        def rctxT(kc):
            return actT[:, kc, NT:NA]

        ctx_all = acc.tile([128, 2, NT], F32, tag="ctx_all")

        # ---- per-core precomputes (one b per core) ----
        # ctx projections, duplicated twice along free dim so a single
        # N=512 identity-matmul injects them into two-target PSUM tiles.
        dups = {}
        for nm, wt in (("kctxT", "kctx_w"), ("vctxT", "vctx_w"),
                       ("dctxT", "dctx_w")):
            dups[nm] = wp.tile([128, 2, C2], F32R, tag=nm, name="dup_" + nm)
            for mc in range(2):
                ps = pp_h.tile([128, C2], F32, tag="h")
                for kc in range(2):
                    nc.tensor.matmul(
                        ps[:, 0:NC],
                        _r(w[wt][:, kc, mc * 128:(mc + 1) * 128]),
                        _r(rctxT(kc)),
                        start=(kc == 0), stop=(kc == 1))
                for rep in range(2):
                    dst = dups[nm][:, mc, rep * NC:(rep + 1) * NC]
                    if mc == 0:
                        nc.scalar.activation(dst, ps[:, 0:NC], AF.Identity)
                    else:
                        nc.vector.tensor_copy(dst, ps[:, 0:NC])

        gctx = wp.tile([128, 2, C2], F32R, tag="gctx")
        for mc in range(2):
            ps = pp_h.tile([128, C2], F32, tag="h")
            i = 0
            for wt, src in (("wg1", "kctxT"), ("wg2", "vctxT")):
                for kc in range(2):
                    nc.tensor.matmul(
                        ps[:, 0:NC],
                        _r(w[wt][:, kc, mc * 128:(mc + 1) * 128]),
                        _r(dups[src][:, kc, 0:NC]),
                        start=(i == 0), stop=(i == 3))
                    i += 1
            for rep in range(2):
                dst = gctx[:, mc, rep * NC:(rep + 1) * NC]
                if mc == 0:
                    nc.scalar.activation(dst, ps[:, 0:NC], AF.Identity)
                else:
                    nc.vector.tensor_copy(dst, ps[:, 0:NC])

        # per-target bias vectors: bias_k = ktgt_w^T R_t^T + b2k, etc.
        bias_t = {}
        for nm, wt, bb in (("bk", "ktgt_w", "b2k"), ("bv", "vtgt_w", "b2v"),
                           ("bkv", "dtgt_w", "db2"), ("q", "wq_s", "bq_s")):
            bias_t[nm] = wp.tile([128, 2, NT], F32R, tag="bt_" + nm,
                                 name="bt_" + nm)
            for mc in range(2):
                ps = pp_h.tile([128, C2], F32, tag="h")
                for kc in range(2):
                    nc.tensor.matmul(
                        ps[:, 0:NT],
                        _r(w[wt][:, kc, mc * 128:(mc + 1) * 128]),
                        _r(rtT(kc)),
                        start=(kc == 0), stop=(kc == 1))
                nc.scalar.activation(
                    bias_t[nm][:, mc, :], ps[:, 0:NT], AF.Identity,
                    bias=w[bb][:, mc:mc + 1])

        # gate bias per target: wg1^T bias_k + wg2^T bias_v + gate_b
        gbias = wp.tile([128, 2, NT], F32, tag="gbias")
        for mc in range(2):
            ps = pp_h.tile([128, C2], F32, tag="h")
            i = 0
            for wt, src in (("wg1", "bk"), ("wg2", "bv")):
                for kc in range(2):
                    nc.tensor.matmul(
                        ps[:, 0:NT],
                        _r(w[wt][:, kc, mc * 128:(mc + 1) * 128]),
                        _r(bias_t[src][:, kc, :]),
                        start=(i == 0), stop=(i == 3))
                    i += 1
            nc.scalar.activation(
                gbias[:, mc, :], ps[:, 0:NT], AF.Identity,
                bias=w["gate_b"][:, mc:mc + 1])

        # ---- supertiles: 2 targets, free dim 512 ----
        # (front halves are queued; back halves are issued one iteration
        # later so each engine always has independent work in flight)
        pending = []

        def drain_one():
            if pending:
                run_back(nc, w, sp, pp_h, pp_big, ctx_all, pending.pop(0))

        for st in range(NST):
            t0 = st * ST_T
            st_state = make_front(nc, w, sp, pp_h, pp_big,
                                  phiT, dups, gctx, bias_t, gbias, t0)
            drain_one()
            pending.append(st_state)

        drain_one()

        # ---- output projection: out^T = out_w^T @ ctx_all + out_b ----
        outT = acc.tile([128, 2, NT], BF16, tag="outT")
        for mc in range(2):
            ps = pp_h.tile([128, C2], F32, tag="h")
            for kc in range(2):
                nc.tensor.matmul(
                    ps[:, 0:NT],
                    _r(w["out_w"][:, kc, mc * 128:(mc + 1) * 128]),
                    _r(ctx_all[:, kc, :]),
                    start=(kc == 0), stop=(kc == 1))
            nc.scalar.activation(outT[:, mc, :], ps[:, 0:NT], AF.Identity,
                                 bias=w["out_b"][:, mc:mc + 1])
        nc.sync.dma_start(out=out_d.ap(), in_=outT[:])

    nc.compile()

    # Normalize per-instruction debug info: recorded tracebacks / absolute
    # file paths otherwise leak the caller's script and kernel.py's location
    # into the serialized BIR, which would make the jax persistent-cache key
    # differ per caller and per checkout path (forcing a spurious recompile).
    def canon(d):
        return type(d)(
            op_name=d.op_name, tensorizer_id=d.tensorizer_id,
            filename="k.py", lineno=0,
            bass_funcname=d.bass_funcname, kernel_name=d.kernel_name,
            ant_traceback=None, ant_layer=d.ant_layer,
            ant_annotation=d.ant_annotation)

    for fn in nc.m.functions:
        for blk in fn.blocks:
            for inst in blk.instructions:
                if inst.debug is not None:
                    inst.debug = canon(inst.debug)
        for alloc in fn.allocations:
            mls = getattr(alloc, "memorylocations", None) or []
            for ml in mls:
                if getattr(ml, "ant_debug", None) is not None:
                    ml.ant_debug = canon(ml.ant_debug)
    return nc


_NC_CACHE = {}
_DISK_DIR = "/tmp/bass_kernel_cache"


class _NcShim:
    """Duck-typed stand-in for the Bass object on run_bass_kernel_spmd's axon
    path: exposes the compiled module plus the handful of attributes the
    bass2jax lowering reads, with to_json_bytes() returning the cached
    serialization (skips re-serializing the module on every call, and lets a
    fresh process skip the whole tile-framework build via the disk cache)."""

    target_bir_lowering = False
    partition_id_tensor = None
    dbg_addr = None
    debug = False
    dbg_callbacks = ()
    has_collectives = False

    class _PidT:
        name = "partition_id"

    def __init__(self, m, json_bytes):
        self.m = m
        self._json = json_bytes
        for alloc in m.functions[0].allocations:
            if (isinstance(alloc, mybir.MemoryLocationSet)
                    and alloc.kind == "ExternalInput"
                    and alloc.memorylocations
                    and alloc.memorylocations[0].name == "partition_id"):
                self.partition_id_tensor = self._PidT()
                break

    def to_json_bytes(self):
        return self._json


def _get_nc(key, inputs):
    if key in _NC_CACHE:
        return _NC_CACHE[key]
    path = os.path.join(_DISK_DIR, key + ".birj")
    shim = None
    if os.path.exists(path):
        try:
            j = open(path, "rb").read()
            shim = _NcShim(mybir.module_from_json_bytes(j), j)
        except Exception:
            shim = None
    if shim is None:
        nc = build_kernel(_marshal_weights(inputs))
        j = nc.to_json_bytes()
        shim = _NcShim(nc.m, j)
        try:
            os.makedirs(_DISK_DIR, exist_ok=True)
            tmp = f"{path}.tmp{os.getpid()}"
            with open(tmp, "wb") as f:
                f.write(j)
            os.replace(tmp, path)
        except Exception:
            pass
    _NC_CACHE[key] = shim
    return shim


_WEIGHT_KEYS = (
    "Wq_w", "Wq_b", "kctx_w", "ktgt_w", "kphi_w1", "kphi_b1", "kphi_w2",
    "kphi_b2", "vctx_w", "vtgt_w", "vphi_w1", "vphi_b1", "vphi_w2", "vphi_b2",
    "gate_w", "gate_b", "out_w", "out_b",
)


_KVER = b"hc-attn-v4"  # bump when build_kernel's emitted program changes


def _whash(inputs):
    h = hashlib.blake2b(digest_size=16)
    h.update(_KVER)
    for k in _WEIGHT_KEYS:
        a = np.ascontiguousarray(np.asarray(inputs[k], np.float32))
        h.update(k.encode())
        h.update(str(a.shape).encode())
        h.update(a.tobytes())
    return h.hexdigest()


def _marshal_weights(inputs):
    f32 = np.float32
    gw = np.asarray(inputs["gate_w"], f32)
    wg1, wg2, wg3 = gw[0:256], gw[256:512], gw[512:768]
    kphi_w2 = np.asarray(inputs["kphi_w2"], f32)
    vphi_w2 = np.asarray(inputs["vphi_w2"], f32)
    sc = 1.0 / np.sqrt(DK)

    mask = np.zeros((256, H), f32)
    for d in range(256):
        mask[d, d // 32] = 1.0
    e_hd = np.ascontiguousarray(mask.T)
    mask_p = _pack(mask)

    wv = {
        "w1k_n": -np.asarray(inputs["kphi_w1"], f32),
        "w1v_n": -np.asarray(inputs["vphi_w1"], f32),
        "b1k": np.asarray(inputs["kphi_b1"], f32).reshape(HID, 1),
        "b1v": np.asarray(inputs["vphi_b1"], f32).reshape(HID, 1),
        "w2k": kphi_w2, "w2v": vphi_w2, "w2v_n": -vphi_w2,
        "kctx_w": _pack(np.asarray(inputs["kctx_w"], f32)),
        "vctx_w": _pack(np.asarray(inputs["vctx_w"], f32)),
        "dctx_w": _pack(np.asarray(inputs["kctx_w"], f32)
                        - np.asarray(inputs["vctx_w"], f32)),
        "wq_s": _pack(np.asarray(inputs["Wq_w"], f32) * sc),
        "bq_s": _packb(np.asarray(inputs["Wq_b"], f32) * sc),
        "ktgt_w": _pack(np.asarray(inputs["ktgt_w"], f32)),
        "vtgt_w": _pack(np.asarray(inputs["vtgt_w"], f32)),
        "dtgt_w": _pack(np.asarray(inputs["ktgt_w"], f32)
                        - np.asarray(inputs["vtgt_w"], f32)),
        "b2k": _packb(np.asarray(inputs["kphi_b2"], f32)),
        "b2v": _packb(np.asarray(inputs["vphi_b2"], f32)),
        "db2": _packb(np.asarray(inputs["kphi_b2"], f32)
                      - np.asarray(inputs["vphi_b2"], f32)),
        "wg1": _pack(wg1), "wg2": _pack(wg2), "wg3": _pack(wg3),
        "wkg1": np.ascontiguousarray(kphi_w2 @ wg1),
        "wvg2": np.ascontiguousarray(vphi_w2 @ wg2),
        "gate_b": _packb(np.asarray(inputs["gate_b"], f32)),
        "out_w": _pack(np.asarray(inputs["out_w"], f32)),
        "out_b": _packb(np.asarray(inputs["out_b"], f32)),
        "mask_qh": mask_p, "e_hd": e_hd, "ident": np.eye(128, dtype=f32),
    }
    return {k: np.ascontiguousarray(
                np.asarray(v, f32).astype(BF16NP) if k in R_NAMES
                else np.asarray(v, f32))
            for k, v in wv.items()}


def kernel(**inputs):
    f32 = np.float32
    # If the caller hands us device-backed (jax) arrays, fetch them all in
    # one batched async device_get -- per-array np.asarray would pay a full
    # relay round-trip each (and np.asarray(x, dtype) can even trigger a
    # device-side convert compile).
    if any(not isinstance(v, np.ndarray) for v in inputs.values()):
        inputs = jax.device_get(inputs)
    key = _whash(inputs)
    nc = _get_nc(key, inputs)

    R_t = np.asarray(inputs["R_t"], f32)
    R_ctx = np.asarray(inputs["R_ctx"], f32)
    phi_t = np.asarray(inputs["phi_t"], f32)
    phi_c = np.asarray(inputs["phi_c"], f32)

    in_maps = []
    for core in range(NCORES):
        b, hh = core // 2, core % 2
        tsl = slice(hh * NT, (hh + 1) * NT)
        rt_p = R_t[b, tsl].T.reshape(2, 128, NT).transpose(1, 0, 2)
        rctx_p = R_ctx[b].T.reshape(2, 128, NC).transpose(1, 0, 2)
        act = np.ascontiguousarray(
            np.concatenate([rt_p, rctx_p], axis=2).astype(BF16NP))
        phi = np.ascontiguousarray(
            np.concatenate([phi_t[b, tsl].T, phi_c[b].T], axis=1))
        in_maps.append({"act": act, "phi": phi})

    res = run_bass_kernel_spmd(nc, in_maps, core_ids=list(range(NCORES)))
    kernel.last_results = res

    out = np.empty((B, NT_FULL, D), f32)
    for core in range(NCORES):
        r = res.results[core]["out_t"]            # (128, 2, 64) bf16
        arr = r.transpose(2, 1, 0).reshape(NT, D)
        b, hh = core // 2, core % 2
        out[b, hh * NT:(hh + 1) * NT, :] = arr.astype(f32)
    return out


# revision 34
# speedup vs baseline: 1.4815x; 1.0707x over previous
"""Bass/Trainium2 kernel for nn_HCTargetAwareAttnNP.

Sharding: 8 cores = B(4) x Nt-half(2). Each core handles one batch b and 64
of the 128 targets, with full R_ctx[b]/phi_c[b] local (softmax over Nc stays
on-core, no collectives).

Host-path design (the wall-clock bottleneck, not device FLOPs):
- All weights are baked into the NEFF as Const tensors (nc.inline_tensor),
  keyed by a hash of the weight inputs -- so per-call PJRT traffic is just
  two small activation tensors per core (~350 KB) instead of ~5 MB of
  replicated weights per core.
- The jax persistent compilation cache is enabled so the per-call
  jax.jit(shard_map(...)) inside run_bass_kernel_spmd deserializes the
  compiled executable instead of re-running XLA/neuronx-cc.

Device layout: everything FEATURE-MAJOR (feature dim on SBUF partitions,
context positions on the free dim); weight matrices are used in native
(in x out) layout as the PE stationary operand, and the pairwise (Nc x D)
tensors per (b,t) are built directly in PSUM by accumulating matmuls. Two
targets per supertile (free dim 512 = 2 x Nc).
"""

import hashlib
import os
import numpy as np
import ml_dtypes
from contextlib import ExitStack

BF16NP = np.float16

import jax

for _k, _v in (
    ("jax_compilation_cache_dir", "/tmp/bass_jax_pcc"),
    ("jax_persistent_cache_min_compile_time_secs", 0.0),
    ("jax_persistent_cache_min_entry_size_bytes", 0),
    # source locations otherwise leak the caller's filename/lineno into the
    # MLIR module, so the persistent-cache key would differ per caller script
    ("jax_include_full_tracebacks_in_locations", False),
    ("jax_traceback_in_locations_limit", 0),
):
    try:
        jax.config.update(_k, _v)
    except Exception:
        pass

import concourse.bass as bass
import concourse.tile as tile
from concourse import bacc, mybir
from concourse.bass_utils import run_bass_kernel_spmd

F32 = mybir.dt.float32
F32R = mybir.dt.float32r
BF16 = mybir.dt.float16
AF = mybir.ActivationFunctionType
ALU = mybir.AluOpType

B, NT_FULL, NC, D, DPHI, HID, H, DK = 4, 128, 256, 256, 16, 128, 8, 32
NCORES = 8
NT = 64                         # local targets per core (half of Nt)
ST_T = 2                        # targets per supertile
C2 = ST_T * NC                  # 512 free dim
NST = NT // ST_T                # 32 supertiles
NA = NT + NC                    # 320 columns in the packed activation tensor

# tensors that feed the PE as lhsT/rhs must be float32r
R_NAMES = {
    "w1k_n", "w1v_n", "w2k", "w2v", "w2v_n",
    "kctx_w", "vctx_w", "dctx_w", "wq_s", "ktgt_w", "vtgt_w", "dtgt_w",
    "wg1", "wg2", "wg3", "wkg1", "wvg2", "mask_qh", "e_hd", "ident",
}


def _r(ap):
    return ap


def _pack(a):
    """(256, M) -> (128, 2, M) with row d at [d % 128, d // 128, :]."""
    m = a.shape[1]
    return np.ascontiguousarray(a.reshape(2, 128, m).transpose(1, 0, 2))


def _packb(a):
    """(256,) -> (128, 2)."""
    return np.ascontiguousarray(a.reshape(2, 128).T)


def make_front(nc, w, sp, pp_h, pp_big, phiT, dups, gctx, bias_t,
               gbias, t0):
    """Issue dphi->h->K/V/D->gate->Kg/Vg for one supertile; returns state for
    the back half (scores/softmax/ctx)."""
    ndphiT = sp.tile([DPHI, C2], F32R, tag="ndphiT", name="ndphiT")
    for ti in range(ST_T):
        nc.vector.tensor_scalar_sub(
            ndphiT[:, ti * NC:(ti + 1) * NC], phiT[:, NT:NA],
            phiT[:, t0 + ti:t0 + ti + 1])

    hs = {}
    for nm in ("k", "v"):
        hps = pp_h.tile([128, C2], F32, tag="h", name="hps_" + nm)
        nc.tensor.matmul(hps[:], w["w1" + nm + "_n"][:], ndphiT[:],
                         start=True, stop=True)
        hs[nm] = sp.tile([128, C2], F32R, tag="h" + nm, name="hs_" + nm)
        nc.scalar.activation(hs[nm][:], hps[:], AF.Relu,
                             bias=w["b1" + nm][:])

    Kp = pp_big.tile([128, 2, C2], F32, tag="big", name="Kp")
    Vp = pp_big.tile([128, 2, C2], F32, tag="big", name="Vp")
    Dp = pp_big.tile([128, 2, C2], F32, tag="big", name="Dp")
    for mc in range(2):
        msl = slice(mc * 128, (mc + 1) * 128)
        nc.tensor.matmul(Kp[:, mc, :], w["w2k"][:, msl], hs["k"][:],
                         start=True, stop=False)
        nc.tensor.matmul(Kp[:, mc, :], w["ident"][:],
                         dups["kctxT"][:, mc, :], start=False, stop=True)
        nc.tensor.matmul(Vp[:, mc, :], w["w2v"][:, msl], hs["v"][:],
                         start=True, stop=False)
        nc.tensor.matmul(Vp[:, mc, :], w["ident"][:],
                         dups["vctxT"][:, mc, :], start=False, stop=True)
        nc.tensor.matmul(Dp[:, mc, :], w["w2k"][:, msl], hs["k"][:],
                         start=True, stop=False)
        nc.tensor.matmul(Dp[:, mc, :], w["w2v_n"][:, msl], hs["v"][:],
                         start=False, stop=False)
        nc.tensor.matmul(Dp[:, mc, :], w["ident"][:],
                         dups["dctxT"][:, mc, :], start=False, stop=True)

    dabs = sp.tile([128, 2, C2], F32R, tag="dabs", name="dabs")
    for mc in range(2):
        for ti in range(ST_T):
            csl = slice(ti * NC, (ti + 1) * NC)
            nc.scalar.activation(
                dabs[:, mc, csl], Dp[:, mc, csl], AF.Abs,
                bias=bias_t["bkv"][:, mc, t0 + ti:t0 + ti + 1].bitcast(F32))

    Gp = pp_big.tile([128, 2, C2], F32, tag="big", name="Gp")
    for mc in range(2):
        msl = slice(mc * 128, (mc + 1) * 128)
        nc.tensor.matmul(Gp[:, mc, :], w["wkg1"][:, msl], hs["k"][:],
                         start=True, stop=False)
        nc.tensor.matmul(Gp[:, mc, :], w["wvg2"][:, msl], hs["v"][:],
                         start=False, stop=False)
        for kc in range(2):
            nc.tensor.matmul(Gp[:, mc, :], w["wg3"][:, kc, msl],
                             dabs[:, kc, :], start=False, stop=False)
        nc.tensor.matmul(Gp[:, mc, :], w["ident"][:], gctx[:, mc, :],
                         start=False, stop=True)

    gs = sp.tile([128, 2, C2], F32, tag="gs", name="gs")
    for mc in range(2):
        for ti in range(ST_T):
            csl = slice(ti * NC, (ti + 1) * NC)
            nc.scalar.activation(
                gs[:, mc, csl], Gp[:, mc, csl], AF.Sigmoid,
                bias=gbias[:, mc, t0 + ti:t0 + ti + 1])

    Kg = sp.tile([128, 2, C2], F32R, tag="Kg", name="Kg")
    Vg = sp.tile([128, 2, C2], F32, tag="Vg", name="Vg")
    for mc in range(2):
        for ti in range(ST_T):
            csl = slice(ti * NC, (ti + 1) * NC)
            nc.vector.scalar_tensor_tensor(
                Kg[:, mc, csl], Kp[:, mc, csl],
                bias_t["bk"][:, mc, t0 + ti:t0 + ti + 1].bitcast(F32),
                gs[:, mc, csl], ALU.add, ALU.mult)
            nc.vector.scalar_tensor_tensor(
                Vg[:, mc, csl], Vp[:, mc, csl],
                bias_t["bv"][:, mc, t0 + ti:t0 + ti + 1].bitcast(F32),
                gs[:, mc, csl], ALU.add, ALU.mult)

    qb = sp.tile([128, 2, ST_T, H], F32R, tag="qb", name="qb")
    for ti in range(ST_T):
        for dc in range(2):
            nc.vector.tensor_scalar_mul(
                qb[:, dc, ti, :], w["mask_qh"][:, dc, :],
                bias_t["q"][:, dc, t0 + ti:t0 + ti + 1].bitcast(F32))
    return (Kg, Vg, qb, t0)


def run_back(nc, w, sp, pp_h, pp_big, ctx_all, state):
    Kg, Vg, qb, col0 = state
    Sps = pp_h.tile([128, C2], F32, tag="h", name="Sps")
    for ti in range(ST_T):
        csl = slice(ti * NC, (ti + 1) * NC)
        for dc in range(2):
            nc.tensor.matmul(Sps[0:H, csl], qb[:, dc, ti, :],
                             Kg[:, dc, csl], start=(dc == 0), stop=(dc == 1))

    attn_u = sp.tile([H, C2], F32, tag="attn_u", name="attn_u")
    rowsum = sp.tile([H, ST_T], F32, tag="rowsum", name="rowsum")
    for ti in range(ST_T):
        csl = slice(ti * NC, (ti + 1) * NC)
        nc.scalar.activation(attn_u[:, csl], Sps[0:H, csl], AF.Exp,
                             accum_out=rowsum[:, ti:ti + 1])
    rsr = sp.tile([H, ST_T], F32, tag="rsr", name="rsr")
    nc.vector.reciprocal(rsr[:], rowsum[:])
    attn_n = sp.tile([H, C2], F32R, tag="attn_n", name="attn_n")
    for ti in range(ST_T):
        csl = slice(ti * NC, (ti + 1) * NC)
        nc.vector.tensor_scalar_mul(attn_n[:, csl], attn_u[:, csl],
                                    rsr[:, ti:ti + 1])

    for dc in range(2):
        Ax = pp_h.tile([128, C2], F32, tag="h", name="Ax")
        nc.tensor.matmul(Ax[:], w["e_hd"][:, dc * 128:(dc + 1) * 128],
                         attn_n[:], start=True, stop=True)
        for ti in range(ST_T):
            csl = slice(ti * NC, (ti + 1) * NC)
            scr = sp.tile([128, NC], F32, tag="scr", name="scr")
            nc.vector.scalar_tensor_tensor(
                scr[:], Vg[:, dc, csl], 0.0, Ax[:, csl],
                ALU.add, ALU.mult,
                accum_out=ctx_all[:, dc, col0 + ti:col0 + ti + 1])


def build_kernel(wv):
    """wv: dict of packed numpy weight arrays; baked into the NEFF as Consts."""
    # disable_frame_to_traceback: recorded tracebacks embed the CALLER's
    # file/line into the BIR debug_table, which leaks into the jax
    # persistent-cache key and forces a recompile per calling script.
    nc = bacc.Bacc("TRN2", target_bir_lowering=False, debug=False,
                   disable_frame_to_traceback=True)

    dr_act = nc.dram_tensor("act", [128, 2, NA], BF16, kind="ExternalInput")
    dr_phi = nc.dram_tensor("phi", [DPHI, NA], F32, kind="ExternalInput")
    out_d = nc.dram_tensor("out_t", [128, 2, NT], BF16, kind="ExternalOutput")

    dr_w = {k: nc.inline_tensor(v, name="cw_" + k) for k, v in wv.items()}

    with ExitStack() as ctx:
        tc = ctx.enter_context(tile.TileContext(nc))
        wp = ctx.enter_context(tc.tile_pool(name="w", bufs=1))
        sp = ctx.enter_context(tc.tile_pool(name="sp", bufs=2))
        acc = ctx.enter_context(tc.tile_pool(name="acc", bufs=1))
        pp_h = ctx.enter_context(
            tc.tile_pool(name="pph", bufs=2, space="PSUM"))
        pp_big = ctx.enter_context(
            tc.tile_pool(name="ppb", bufs=3, space="PSUM"))

        # bf16 consts are DMA'd into bf16 staging tiles, then upcast into the
        # float32r tiles the PE consumes (alternating engines for overlap).
        w = {}
        upcast_i = 0
        for k, v in wv.items():
            if v.dtype == BF16NP:
                stg = wp.tile(list(v.shape), BF16, tag="s_" + k,
                              name="s_" + k)
                nc.sync.dma_start(out=stg[:], in_=dr_w[k].ap())
                w[k] = wp.tile(list(v.shape), F32R, tag=k, name="w_" + k)
                if upcast_i % 2 == 0:
                    nc.vector.tensor_copy(w[k][:], stg[:])
                else:
                    nc.scalar.activation(w[k][:], stg[:], AF.Identity)
                upcast_i += 1
            else:
                w[k] = wp.tile(list(v.shape), F32, tag=k, name="w_" + k)
                nc.sync.dma_start(out=w[k][:], in_=dr_w[k].ap())

        actS = wp.tile([128, 2, NA], BF16, tag="actS", name="actS")
        nc.sync.dma_start(out=actS[:], in_=dr_act.ap())
        actT = wp.tile([128, 2, NA], F32R, tag="actT", name="actT")
        nc.vector.tensor_copy(actT[:], actS[:])
        phiT = wp.tile([DPHI, NA], F32, tag="phiT", name="phiT")
        nc.sync.dma_start(out=phiT[:], in_=dr_phi.ap())

        def rtT(kc):
            return actT[:, kc, 0:NT]

        def rctxT(kc):
            return actT[:, kc, NT:NA]

        ctx_all = acc.tile([128, 2, NT], F32, tag="ctx_all")

        # ---- per-core precomputes (one b per core) ----
        # ctx projections, duplicated twice along free dim so a single
        # N=512 identity-matmul injects them into two-target PSUM tiles.
        dups = {}
        for nm, wt in (("kctxT", "kctx_w"), ("vctxT", "vctx_w"),
                       ("dctxT", "dctx_w")):
            dups[nm] = wp.tile([128, 2, C2], F32R, tag=nm, name="dup_" + nm)
            for mc in range(2):
                ps = pp_h.tile([128, C2], F32, tag="h")
                for kc in range(2):
                    nc.tensor.matmul(
                        ps[:, 0:NC],
                        _r(w[wt][:, kc, mc * 128:(mc + 1) * 128]),
                        _r(rctxT(kc)),
                        start=(kc == 0), stop=(kc == 1))
                for rep in range(2):
                    dst = dups[nm][:, mc, rep * NC:(rep + 1) * NC]
                    if mc == 0:
                        nc.scalar.activation(dst, ps[:, 0:NC], AF.Identity)
                    else:
                        nc.vector.tensor_copy(dst, ps[:, 0:NC])

        gctx = wp.tile([128, 2, C2], F32R, tag="gctx")
        for mc in range(2):
            ps = pp_h.tile([128, C2], F32, tag="h")
            i = 0
            for wt, src in (("wg1", "kctxT"), ("wg2", "vctxT")):
                for kc in range(2):
                    nc.tensor.matmul(
                        ps[:, 0:NC],
                        _r(w[wt][:, kc, mc * 128:(mc + 1) * 128]),
                        _r(dups[src][:, kc, 0:NC]),
                        start=(i == 0), stop=(i == 3))
                    i += 1
            for rep in range(2):
                dst = gctx[:, mc, rep * NC:(rep + 1) * NC]
                if mc == 0:
                    nc.scalar.activation(dst, ps[:, 0:NC], AF.Identity)
                else:
                    nc.vector.tensor_copy(dst, ps[:, 0:NC])

        # per-target bias vectors: bias_k = ktgt_w^T R_t^T + b2k, etc.
        bias_t = {}
        for nm, wt, bb in (("bk", "ktgt_w", "b2k"), ("bv", "vtgt_w", "b2v"),
                           ("bkv", "dtgt_w", "db2"), ("q", "wq_s", "bq_s")):
            bias_t[nm] = wp.tile([128, 2, NT], F32R, tag="bt_" + nm,
                                 name="bt_" + nm)
            for mc in range(2):
                ps = pp_h.tile([128, C2], F32, tag="h")
                for kc in range(2):
                    nc.tensor.matmul(
                        ps[:, 0:NT],
                        _r(w[wt][:, kc, mc * 128:(mc + 1) * 128]),
                        _r(rtT(kc)),
                        start=(kc == 0), stop=(kc == 1))
                nc.scalar.activation(
                    bias_t[nm][:, mc, :], ps[:, 0:NT], AF.Identity,
                    bias=w[bb][:, mc:mc + 1])

        # gate bias per target: wg1^T bias_k + wg2^T bias_v + gate_b
        gbias = wp.tile([128, 2, NT], F32, tag="gbias")
        for mc in range(2):
            ps = pp_h.tile([128, C2], F32, tag="h")
            i = 0
            for wt, src in (("wg1", "bk"), ("wg2", "bv")):
                for kc in range(2):
                    nc.tensor.matmul(
                        ps[:, 0:NT],
                        _r(w[wt][:, kc, mc * 128:(mc + 1) * 128]),
                        _r(bias_t[src][:, kc, :]),
                        start=(i == 0), stop=(i == 3))
                    i += 1
            nc.scalar.activation(
                gbias[:, mc, :], ps[:, 0:NT], AF.Identity,
                bias=w["gate_b"][:, mc:mc + 1])

        # ---- supertiles: 2 targets, free dim 512 ----
        # (front halves are queued; back halves are issued one iteration
        # later so each engine always has independent work in flight)
        pending = []

        def drain_one():
            if pending:
                run_back(nc, w, sp, pp_h, pp_big, ctx_all, pending.pop(0))

        for st in range(NST):
            t0 = st * ST_T
            st_state = make_front(nc, w, sp, pp_h, pp_big,
                                  phiT, dups, gctx, bias_t, gbias, t0)
            drain_one()
            pending.append(st_state)

        drain_one()

        # ---- output projection: out^T = out_w^T @ ctx_all + out_b ----
        outT = acc.tile([128, 2, NT], BF16, tag="outT")
        for mc in range(2):
            ps = pp_h.tile([128, C2], F32, tag="h")
            for kc in range(2):
                nc.tensor.matmul(
                    ps[:, 0:NT],
                    _r(w["out_w"][:, kc, mc * 128:(mc + 1) * 128]),
                    _r(ctx_all[:, kc, :]),
                    start=(kc == 0), stop=(kc == 1))
            nc.scalar.activation(outT[:, mc, :], ps[:, 0:NT], AF.Identity,
                                 bias=w["out_b"][:, mc:mc + 1])
        nc.sync.dma_start(out=out_d.ap(), in_=outT[:])

    nc.compile()

    # Normalize per-instruction debug info: recorded tracebacks / absolute
    # file paths otherwise leak the caller's script and kernel.py's location
    # into the serialized BIR, which would make the jax persistent-cache key
    # differ per caller and per checkout path (forcing a spurious recompile).
    def canon(d):
        return type(d)(
            op_name=d.op_name, tensorizer_id=d.tensorizer_id,
            filename="k.py", lineno=0,
            bass_funcname=d.bass_funcname, kernel_name=d.kernel_name,
            ant_traceback=None, ant_layer=d.ant_layer,
            ant_annotation=d.ant_annotation)

    for fn in nc.m.functions:
        for blk in fn.blocks:
            for inst in blk.instructions:
                if inst.debug is not None:
                    inst.debug = canon(inst.debug)
        for alloc in fn.allocations:
            mls = getattr(alloc, "memorylocations", None) or []
            for ml in mls:
                if getattr(ml, "ant_debug", None) is not None:
                    ml.ant_debug = canon(ml.ant_debug)
    return nc


_NC_CACHE = {}
_DISK_DIR = "/tmp/bass_kernel_cache"


class _NcShim:
    """Duck-typed stand-in for the Bass object on run_bass_kernel_spmd's axon
    path: exposes the compiled module plus the handful of attributes the
    bass2jax lowering reads, with to_json_bytes() returning the cached
    serialization (skips re-serializing the module on every call, and lets a
    fresh process skip the whole tile-framework build via the disk cache)."""

    target_bir_lowering = False
    partition_id_tensor = None
    dbg_addr = None
    debug = False
    dbg_callbacks = ()
    has_collectives = False

    class _PidT:
        name = "partition_id"

    def __init__(self, m, json_bytes):
        self.m = m
        self._json = json_bytes
        for alloc in m.functions[0].allocations:
            if (isinstance(alloc, mybir.MemoryLocationSet)
                    and alloc.kind == "ExternalInput"
                    and alloc.memorylocations
                    and alloc.memorylocations[0].name == "partition_id"):
                self.partition_id_tensor = self._PidT()
                break

    def to_json_bytes(self):
        return self._json


def _get_nc(key, inputs):
    if key in _NC_CACHE:
        return _NC_CACHE[key]
    path = os.path.join(_DISK_DIR, key + ".birj")
    shim = None
    if os.path.exists(path):
        try:
            j = open(path, "rb").read()
            shim = _NcShim(mybir.module_from_json_bytes(j), j)
        except Exception:
            shim = None
    if shim is None:
        nc = build_kernel(_marshal_weights(inputs))
        j = nc.to_json_bytes()
        shim = _NcShim(nc.m, j)
        try:
            os.makedirs(_DISK_DIR, exist_ok=True)
            tmp = f"{path}.tmp{os.getpid()}"
            with open(tmp, "wb") as f:
                f.write(j)
            os.replace(tmp, path)
        except Exception:
            pass
    _NC_CACHE[key] = shim
    return shim


_WEIGHT_KEYS = (
    "Wq_w", "Wq_b", "kctx_w", "ktgt_w", "kphi_w1", "kphi_b1", "kphi_w2",
    "kphi_b2", "vctx_w", "vtgt_w", "vphi_w1", "vphi_b1", "vphi_w2", "vphi_b2",
    "gate_w", "gate_b", "out_w", "out_b",
)


_KVER = b"hc-attn-v4"  # bump when build_kernel's emitted program changes


def _whash(inputs):
    h = hashlib.blake2b(digest_size=16)
    h.update(_KVER)
    for k in _WEIGHT_KEYS:
        a = np.ascontiguousarray(np.asarray(inputs[k], np.float32))
        h.update(k.encode())
        h.update(str(a.shape).encode())
        h.update(a.tobytes())
    return h.hexdigest()


def _marshal_weights(inputs):
    f32 = np.float32
    gw = np.asarray(inputs["gate_w"], f32)
    wg1, wg2, wg3 = gw[0:256], gw[256:512], gw[512:768]
    kphi_w2 = np.asarray(inputs["kphi_w2"], f32)
    vphi_w2 = np.asarray(inputs["vphi_w2"], f32)
    sc = 1.0 / np.sqrt(DK)

    mask = np.zeros((256, H), f32)
    for d in range(256):
        mask[d, d // 32] = 1.0
    e_hd = np.ascontiguousarray(mask.T)
    mask_p = _pack(mask)

    wv = {
        "w1k_n": -np.asarray(inputs["kphi_w1"], f32),
        "w1v_n": -np.asarray(inputs["vphi_w1"], f32),
        "b1k": np.asarray(inputs["kphi_b1"], f32).reshape(HID, 1),
        "b1v": np.asarray(inputs["vphi_b1"], f32).reshape(HID, 1),
        "w2k": kphi_w2, "w2v": vphi_w2, "w2v_n": -vphi_w2,
        "kctx_w": _pack(np.asarray(inputs["kctx_w"], f32)),
        "vctx_w": _pack(np.asarray(inputs["vctx_w"], f32)),
        "dctx_w": _pack(np.asarray(inputs["kctx_w"], f32)
                        - np.asarray(inputs["vctx_w"], f32)),
        "wq_s": _pack(np.asarray(inputs["Wq_w"], f32) * sc),
        "bq_s": _packb(np.asarray(inputs["Wq_b"], f32) * sc),
        "ktgt_w": _pack(np.asarray(inputs["ktgt_w"], f32)),
        "vtgt_w": _pack(np.asarray(inputs["vtgt_w"], f32)),
        "dtgt_w": _pack(np.asarray(inputs["ktgt_w"], f32)
                        - np.asarray(inputs["vtgt_w"], f32)),
        "b2k": _packb(np.asarray(inputs["kphi_b2"], f32)),
        "b2v": _packb(np.asarray(inputs["vphi_b2"], f32)),
        "db2": _packb(np.asarray(inputs["kphi_b2"], f32)
                      - np.asarray(inputs["vphi_b2"], f32)),
        "wg1": _pack(wg1), "wg2": _pack(wg2), "wg3": _pack(wg3),
        "wkg1": np.ascontiguousarray(kphi_w2 @ wg1),
        "wvg2": np.ascontiguousarray(vphi_w2 @ wg2),
        "gate_b": _packb(np.asarray(inputs["gate_b"], f32)),
        "out_w": _pack(np.asarray(inputs["out_w"], f32)),
        "out_b": _packb(np.asarray(inputs["out_b"], f32)),
        "mask_qh": mask_p, "e_hd": e_hd, "ident": np.eye(128, dtype=f32),
    }
    return {k: np.ascontiguousarray(
                np.asarray(v, f32).astype(BF16NP) if k in R_NAMES
                else np.asarray(v, f32))
            for k, v in wv.items()}


def kernel(**inputs):
    f32 = np.float32
    # If the caller hands us device-backed (jax) arrays, fetch them all in
    # one batched async device_get -- per-array np.asarray would pay a full
    # relay round-trip each (and np.asarray(x, dtype) can even trigger a
    # device-side convert compile).
    if any(not isinstance(v, np.ndarray) for v in inputs.values()):
        inputs = jax.device_get(inputs)
    key = _whash(inputs)
    nc = _get_nc(key, inputs)

    R_t = np.asarray(inputs["R_t"], f32)
    R_ctx = np.asarray(inputs["R_ctx"], f32)
    phi_t = np.asarray(inputs["phi_t"], f32)
    phi_c = np.asarray(inputs["phi_c"], f32)

    in_maps = []
    for core in range(NCORES):
        b, hh = core // 2, core % 2
        tsl = slice(hh * NT, (hh + 1) * NT)
        rt_p = R_t[b, tsl].T.reshape(2, 128, NT).transpose(1, 0, 2)
        rctx_p = R_ctx[b].T.reshape(2, 128, NC).transpose(1, 0, 2)
        act = np.ascontiguousarray(
            np.concatenate([rt_p, rctx_p], axis=2).astype(BF16NP))
        phi = np.ascontiguousarray(
            np.concatenate([phi_t[b, tsl].T, phi_c[b].T], axis=1))
        in_maps.append({"act": act, "phi": phi})

    res = run_bass_kernel_spmd(nc, in_maps, core_ids=list(range(NCORES)))
    kernel.last_results = res

    out = np.empty((B, NT_FULL, D), f32)
    for core in range(NCORES):
        r = res.results[core]["out_t"]            # (128, 2, 64) bf16
        arr = r.transpose(2, 1, 0).reshape(NT, D)
        b, hh = core // 2, core % 2
        out[b, hh * NT:(hh + 1) * NT, :] = arr.astype(f32)
    return out


# revision 36
# speedup vs baseline: 1.7285x; 1.1667x over previous
"""Bass/Trainium2 kernel for nn_HCTargetAwareAttnNP.

Sharding: 8 cores = B(4) x Nt-half(2). Each core handles one batch b and 64
of the 128 targets, with full R_ctx[b]/phi_c[b] local (softmax over Nc stays
on-core, no collectives).

Host-path design (the wall-clock bottleneck, not device FLOPs):
- All weights are baked into the NEFF as Const tensors (nc.inline_tensor),
  keyed by a hash of the weight inputs -- so per-call PJRT traffic is just
  two small activation tensors per core (~350 KB) instead of ~5 MB of
  replicated weights per core.
- The jax persistent compilation cache is enabled so the per-call
  jax.jit(shard_map(...)) inside run_bass_kernel_spmd deserializes the
  compiled executable instead of re-running XLA/neuronx-cc.

Device layout: everything FEATURE-MAJOR (feature dim on SBUF partitions,
context positions on the free dim); weight matrices are used in native
(in x out) layout as the PE stationary operand, and the pairwise (Nc x D)
tensors per (b,t) are built directly in PSUM by accumulating matmuls. Two
targets per supertile (free dim 512 = 2 x Nc).
"""

import hashlib
import os
import numpy as np
import ml_dtypes
from contextlib import ExitStack

BF16NP = np.float16

import jax

for _k, _v in (
    ("jax_compilation_cache_dir", "/tmp/bass_jax_pcc"),
    ("jax_persistent_cache_min_compile_time_secs", 0.0),
    ("jax_persistent_cache_min_entry_size_bytes", 0),
    # source locations otherwise leak the caller's filename/lineno into the
    # MLIR module, so the persistent-cache key would differ per caller script
    ("jax_include_full_tracebacks_in_locations", False),
    ("jax_traceback_in_locations_limit", 0),
):
    try:
        jax.config.update(_k, _v)
    except Exception:
        pass

import concourse.bass as bass
from concourse.bass import ds
import concourse.tile as tile
from concourse import bacc, mybir
from concourse.bass_utils import run_bass_kernel_spmd

F32 = mybir.dt.float32
F32R = mybir.dt.float32r
BF16 = mybir.dt.float16
AF = mybir.ActivationFunctionType
ALU = mybir.AluOpType

B, NT_FULL, NC, D, DPHI, HID, H, DK = 4, 128, 256, 256, 16, 128, 8, 32
NCORES = 8
NT = 64                         # local targets per core (half of Nt)
ST_T = 2                        # targets per supertile
C2 = ST_T * NC                  # 512 free dim
NST = NT // ST_T                # 32 supertiles
NA = NT + NC                    # 320 columns in the packed activation tensor

# tensors that feed the PE as lhsT/rhs must be float32r
R_NAMES = {
    "w1k_n", "w1v_n", "w2k", "w2v", "w2v_n",
    "kctx_w", "vctx_w", "dctx_w", "wq_s", "ktgt_w", "vtgt_w", "dtgt_w",
    "wg1", "wg2", "wg3", "wkg1", "wvg2", "mask_qh", "e_hd", "ident",
}


def _r(ap):
    return ap


def _pack(a):
    """(256, M) -> (128, 2, M) with row d at [d % 128, d // 128, :]."""
    m = a.shape[1]
    return np.ascontiguousarray(a.reshape(2, 128, m).transpose(1, 0, 2))


def _packb(a):
    """(256,) -> (128, 2)."""
    return np.ascontiguousarray(a.reshape(2, 128).T)


def make_front(nc, w, sp, pp_h, pp_big, phiT, dups, gctx, bias_t,
               gbias, t0):
    """Issue dphi->h->K/V/D->gate->Kg/Vg for one supertile; returns state for
    the back half (scores/softmax/ctx)."""
    pc = sp.tile([DPHI, ST_T], F32, tag="pc", name="pc")
    nc.vector.tensor_copy(pc[:], phiT[:, ds(t0, ST_T)])
    bst = {}
    for nm in ("bk", "bv", "bkv", "q"):
        bst[nm] = sp.tile([128, 2, ST_T], F32R, tag="bst_" + nm,
                          name="bst_" + nm)
        nc.vector.tensor_copy(bst[nm][:], bias_t[nm][:, :, ds(t0, ST_T)])
    gbst = sp.tile([128, 2, ST_T], F32, tag="gbst", name="gbst")
    nc.scalar.activation(gbst[:], gbias[:, :, ds(t0, ST_T)], AF.Identity)

    ndphiT = sp.tile([DPHI, C2], F32R, tag="ndphiT", name="ndphiT")
    for ti in range(ST_T):
        nc.vector.tensor_scalar_sub(
            ndphiT[:, ti * NC:(ti + 1) * NC], phiT[:, NT:NA],
            pc[:, ti:ti + 1])

    hs = {}
    for nm in ("k", "v"):
        hps = pp_h.tile([128, C2], F32, tag="h", name="hps_" + nm)
        nc.tensor.matmul(hps[:], w["w1" + nm + "_n"][:], ndphiT[:],
                         start=True, stop=True)
        hs[nm] = sp.tile([128, C2], F32R, tag="h" + nm, name="hs_" + nm)
        nc.scalar.activation(hs[nm][:], hps[:], AF.Relu,
                             bias=w["b1" + nm][:])

    Kp = pp_big.tile([128, 2, C2], F32, tag="big", name="Kp")
    Vp = pp_big.tile([128, 2, C2], F32, tag="big", name="Vp")
    Dp = pp_big.tile([128, 2, C2], F32, tag="big", name="Dp")
    for mc in range(2):
        msl = slice(mc * 128, (mc + 1) * 128)
        nc.tensor.matmul(Kp[:, mc, :], w["w2k"][:, msl], hs["k"][:],
                         start=True, stop=False)
        nc.tensor.matmul(Kp[:, mc, :], w["ident"][:],
                         dups["kctxT"][:, mc, :], start=False, stop=True)
        nc.tensor.matmul(Vp[:, mc, :], w["w2v"][:, msl], hs["v"][:],
                         start=True, stop=False)
        nc.tensor.matmul(Vp[:, mc, :], w["ident"][:],
                         dups["vctxT"][:, mc, :], start=False, stop=True)
        nc.tensor.matmul(Dp[:, mc, :], w["w2k"][:, msl], hs["k"][:],
                         start=True, stop=False)
        nc.tensor.matmul(Dp[:, mc, :], w["w2v_n"][:, msl], hs["v"][:],
                         start=False, stop=False)
        nc.tensor.matmul(Dp[:, mc, :], w["ident"][:],
                         dups["dctxT"][:, mc, :], start=False, stop=True)

    dabs = sp.tile([128, 2, C2], F32R, tag="dabs", name="dabs")
    for mc in range(2):
        for ti in range(ST_T):
            csl = slice(ti * NC, (ti + 1) * NC)
            nc.scalar.activation(
                dabs[:, mc, csl], Dp[:, mc, csl], AF.Abs,
                bias=bst["bkv"][:, mc, ti:ti + 1].bitcast(F32))

    Gp = pp_big.tile([128, 2, C2], F32, tag="big", name="Gp")
    for mc in range(2):
        msl = slice(mc * 128, (mc + 1) * 128)
        nc.tensor.matmul(Gp[:, mc, :], w["wkg1"][:, msl], hs["k"][:],
                         start=True, stop=False)
        nc.tensor.matmul(Gp[:, mc, :], w["wvg2"][:, msl], hs["v"][:],
                         start=False, stop=False)
        for kc in range(2):
            nc.tensor.matmul(Gp[:, mc, :], w["wg3"][:, kc, msl],
                             dabs[:, kc, :], start=False, stop=False)
        nc.tensor.matmul(Gp[:, mc, :], w["ident"][:], gctx[:, mc, :],
                         start=False, stop=True)

    gs = sp.tile([128, 2, C2], F32, tag="gs", name="gs")
    for mc in range(2):
        for ti in range(ST_T):
            csl = slice(ti * NC, (ti + 1) * NC)
            nc.scalar.activation(
                gs[:, mc, csl], Gp[:, mc, csl], AF.Sigmoid,
                bias=gbst[:, mc, ti:ti + 1])

    Kg = sp.tile([128, 2, C2], F32R, tag="Kg", name="Kg")
    Vg = sp.tile([128, 2, C2], F32, tag="Vg", name="Vg")
    for mc in range(2):
        for ti in range(ST_T):
            csl = slice(ti * NC, (ti + 1) * NC)
            nc.vector.scalar_tensor_tensor(
                Kg[:, mc, csl], Kp[:, mc, csl],
                bst["bk"][:, mc, ti:ti + 1].bitcast(F32),
                gs[:, mc, csl], ALU.add, ALU.mult)
            nc.vector.scalar_tensor_tensor(
                Vg[:, mc, csl], Vp[:, mc, csl],
                bst["bv"][:, mc, ti:ti + 1].bitcast(F32),
                gs[:, mc, csl], ALU.add, ALU.mult)

    qb = sp.tile([128, 2, ST_T, H], F32R, tag="qb", name="qb")
    for ti in range(ST_T):
        for dc in range(2):
            nc.vector.tensor_scalar_mul(
                qb[:, dc, ti, :], w["mask_qh"][:, dc, :],
                bst["q"][:, dc, ti:ti + 1].bitcast(F32))
    return (Kg, Vg, qb, t0)


def run_back(nc, w, sp, pp_h, pp_big, ctx_all, state):
    Kg, Vg, qb, col0 = state
    Sps = pp_h.tile([128, C2], F32, tag="h", name="Sps")
    for ti in range(ST_T):
        csl = slice(ti * NC, (ti + 1) * NC)
        for dc in range(2):
            nc.tensor.matmul(Sps[0:H, csl], qb[:, dc, ti, :],
                             Kg[:, dc, csl], start=(dc == 0), stop=(dc == 1))

    attn_u = sp.tile([H, C2], F32, tag="attn_u", name="attn_u")
    rowsum = sp.tile([H, ST_T], F32, tag="rowsum", name="rowsum")
    for ti in range(ST_T):
        csl = slice(ti * NC, (ti + 1) * NC)
        nc.scalar.activation(attn_u[:, csl], Sps[0:H, csl], AF.Exp,
                             accum_out=rowsum[:, ti:ti + 1])
    rsr = sp.tile([H, ST_T], F32, tag="rsr", name="rsr")
    nc.vector.reciprocal(rsr[:], rowsum[:])
    attn_n = sp.tile([H, C2], F32R, tag="attn_n", name="attn_n")
    for ti in range(ST_T):
        csl = slice(ti * NC, (ti + 1) * NC)
        nc.vector.tensor_scalar_mul(attn_n[:, csl], attn_u[:, csl],
                                    rsr[:, ti:ti + 1])

    ccols = sp.tile([128, 2, ST_T], F32, tag="ccols", name="ccols")
    for dc in range(2):
        Ax = pp_h.tile([128, C2], F32, tag="h", name="Ax")
        nc.tensor.matmul(Ax[:], w["e_hd"][:, dc * 128:(dc + 1) * 128],
                         attn_n[:], start=True, stop=True)
        for ti in range(ST_T):
            csl = slice(ti * NC, (ti + 1) * NC)
            scr = sp.tile([128, NC], F32, tag="scr", name="scr")
            nc.vector.scalar_tensor_tensor(
                scr[:], Vg[:, dc, csl], 0.0, Ax[:, csl],
                ALU.add, ALU.mult,
                accum_out=ccols[:, dc, ti:ti + 1])
    for dc in range(2):
        nc.vector.tensor_copy(ctx_all[:, dc, ds(col0, ST_T)],
                              ccols[:, dc, :])


def build_kernel(wv):
    """wv: dict of packed numpy weight arrays; baked into the NEFF as Consts."""
    # disable_frame_to_traceback: recorded tracebacks embed the CALLER's
    # file/line into the BIR debug_table, which leaks into the jax
    # persistent-cache key and forces a recompile per calling script.
    nc = bacc.Bacc("TRN2", target_bir_lowering=False, debug=False,
                   disable_frame_to_traceback=True)

    dr_act = nc.dram_tensor("act", [128, 2, NA], BF16, kind="ExternalInput")
    dr_phi = nc.dram_tensor("phi", [DPHI, NA], F32, kind="ExternalInput")
    out_d = nc.dram_tensor("out_t", [128, 2, NT], BF16, kind="ExternalOutput")

    dr_w = {k: nc.inline_tensor(v, name="cw_" + k) for k, v in wv.items()}

    with ExitStack() as ctx:
        tc = ctx.enter_context(tile.TileContext(nc))
        wp = ctx.enter_context(tc.tile_pool(name="w", bufs=1))
        sp = ctx.enter_context(tc.tile_pool(name="sp", bufs=2))
        acc = ctx.enter_context(tc.tile_pool(name="acc", bufs=1))
        pp_h = ctx.enter_context(
            tc.tile_pool(name="pph", bufs=2, space="PSUM"))
        pp_big = ctx.enter_context(
            tc.tile_pool(name="ppb", bufs=3, space="PSUM"))

        # bf16 consts are DMA'd into bf16 staging tiles, then upcast into the
        # float32r tiles the PE consumes (alternating engines for overlap).
        w = {}
        upcast_i = 0
        for k, v in wv.items():
            if v.dtype == BF16NP:
                stg = wp.tile(list(v.shape), BF16, tag="s_" + k,
                              name="s_" + k)
                nc.sync.dma_start(out=stg[:], in_=dr_w[k].ap())
                w[k] = wp.tile(list(v.shape), F32R, tag=k, name="w_" + k)
                if upcast_i % 2 == 0:
                    nc.vector.tensor_copy(w[k][:], stg[:])
                else:
                    nc.scalar.activation(w[k][:], stg[:], AF.Identity)
                upcast_i += 1
            else:
                w[k] = wp.tile(list(v.shape), F32, tag=k, name="w_" + k)
                nc.sync.dma_start(out=w[k][:], in_=dr_w[k].ap())

        actS = wp.tile([128, 2, NA], BF16, tag="actS", name="actS")
        nc.sync.dma_start(out=actS[:], in_=dr_act.ap())
        actT = wp.tile([128, 2, NA], F32R, tag="actT", name="actT")
        nc.vector.tensor_copy(actT[:], actS[:])
        phiT = wp.tile([DPHI, NA], F32, tag="phiT", name="phiT")
        nc.sync.dma_start(out=phiT[:], in_=dr_phi.ap())

        def rtT(kc):
            return actT[:, kc, 0:NT]

        def rctxT(kc):
            return actT[:, kc, NT:NA]

        ctx_all = acc.tile([128, 2, NT], F32, tag="ctx_all")

        # ---- per-core precomputes (one b per core) ----
        # ctx projections, duplicated twice along free dim so a single
        # N=512 identity-matmul injects them into two-target PSUM tiles.
        dups = {}
        for nm, wt in (("kctxT", "kctx_w"), ("vctxT", "vctx_w"),
                       ("dctxT", "dctx_w")):
            dups[nm] = wp.tile([128, 2, C2], F32R, tag=nm, name="dup_" + nm)
            for mc in range(2):
                ps = pp_h.tile([128, C2], F32, tag="h")
                for kc in range(2):
                    nc.tensor.matmul(
                        ps[:, 0:NC],
                        _r(w[wt][:, kc, mc * 128:(mc + 1) * 128]),
                        _r(rctxT(kc)),
                        start=(kc == 0), stop=(kc == 1))
                for rep in range(2):
                    dst = dups[nm][:, mc, rep * NC:(rep + 1) * NC]
                    if mc == 0:
                        nc.scalar.activation(dst, ps[:, 0:NC], AF.Identity)
                    else:
                        nc.vector.tensor_copy(dst, ps[:, 0:NC])

        gctx = wp.tile([128, 2, C2], F32R, tag="gctx")
        for mc in range(2):
            ps = pp_h.tile([128, C2], F32, tag="h")
            i = 0
            for wt, src in (("wg1", "kctxT"), ("wg2", "vctxT")):
                for kc in range(2):
                    nc.tensor.matmul(
                        ps[:, 0:NC],
                        _r(w[wt][:, kc, mc * 128:(mc + 1) * 128]),
                        _r(dups[src][:, kc, 0:NC]),
                        start=(i == 0), stop=(i == 3))
                    i += 1
            for rep in range(2):
                dst = gctx[:, mc, rep * NC:(rep + 1) * NC]
                if mc == 0:
                    nc.scalar.activation(dst, ps[:, 0:NC], AF.Identity)
                else:
                    nc.vector.tensor_copy(dst, ps[:, 0:NC])

        # per-target bias vectors: bias_k = ktgt_w^T R_t^T + b2k, etc.
        bias_t = {}
        for nm, wt, bb in (("bk", "ktgt_w", "b2k"), ("bv", "vtgt_w", "b2v"),
                           ("bkv", "dtgt_w", "db2"), ("q", "wq_s", "bq_s")):
            bias_t[nm] = wp.tile([128, 2, NT], F32R, tag="bt_" + nm,
                                 name="bt_" + nm)
            for mc in range(2):
                ps = pp_h.tile([128, C2], F32, tag="h")
                for kc in range(2):
                    nc.tensor.matmul(
                        ps[:, 0:NT],
                        _r(w[wt][:, kc, mc * 128:(mc + 1) * 128]),
                        _r(rtT(kc)),
                        start=(kc == 0), stop=(kc == 1))
                nc.scalar.activation(
                    bias_t[nm][:, mc, :], ps[:, 0:NT], AF.Identity,
                    bias=w[bb][:, mc:mc + 1])

        # gate bias per target: wg1^T bias_k + wg2^T bias_v + gate_b
        gbias = wp.tile([128, 2, NT], F32, tag="gbias")
        for mc in range(2):
            ps = pp_h.tile([128, C2], F32, tag="h")
            i = 0
            for wt, src in (("wg1", "bk"), ("wg2", "bv")):
                for kc in range(2):
                    nc.tensor.matmul(
                        ps[:, 0:NT],
                        _r(w[wt][:, kc, mc * 128:(mc + 1) * 128]),
                        _r(bias_t[src][:, kc, :]),
                        start=(i == 0), stop=(i == 3))
                    i += 1
            nc.scalar.activation(
                gbias[:, mc, :], ps[:, 0:NT], AF.Identity,
                bias=w["gate_b"][:, mc:mc + 1])

        # ---- supertiles: 2 targets, free dim 512, hardware loop ----
        # Per-target bias columns are staged into static tiles via copies
        # whose main APs carry the dynamic ds() slice -- auxiliary operands
        # (bias=, scalar, accum_out) do not honor the loop IV.
        with tc.For_i(0, NT, ST_T) as iv:
            st_state = make_front(nc, w, sp, pp_h, pp_big,
                                  phiT, dups, gctx, bias_t, gbias, iv)
            run_back(nc, w, sp, pp_h, pp_big, ctx_all, st_state)

        # ---- output projection: out^T = out_w^T @ ctx_all + out_b ----
        outT = acc.tile([128, 2, NT], BF16, tag="outT")
        for mc in range(2):
            ps = pp_h.tile([128, C2], F32, tag="h")
            for kc in range(2):
                nc.tensor.matmul(
                    ps[:, 0:NT],
                    _r(w["out_w"][:, kc, mc * 128:(mc + 1) * 128]),
                    _r(ctx_all[:, kc, :]),
                    start=(kc == 0), stop=(kc == 1))
            nc.scalar.activation(outT[:, mc, :], ps[:, 0:NT], AF.Identity,
                                 bias=w["out_b"][:, mc:mc + 1])
        nc.sync.dma_start(out=out_d.ap(), in_=outT[:])

    nc.compile()

    # Normalize per-instruction debug info: recorded tracebacks / absolute
    # file paths otherwise leak the caller's script and kernel.py's location
    # into the serialized BIR, which would make the jax persistent-cache key
    # differ per caller and per checkout path (forcing a spurious recompile).
    def canon(d):
        return type(d)(
            op_name=d.op_name, tensorizer_id=d.tensorizer_id,
            filename="k.py", lineno=0,
            bass_funcname=d.bass_funcname, kernel_name=d.kernel_name,
            ant_traceback=None, ant_layer=d.ant_layer,
            ant_annotation=d.ant_annotation)

    for fn in nc.m.functions:
        for blk in fn.blocks:
            for inst in blk.instructions:
                if inst.debug is not None:
                    inst.debug = canon(inst.debug)
        for alloc in fn.allocations:
            mls = getattr(alloc, "memorylocations", None) or []
            for ml in mls:
                if getattr(ml, "ant_debug", None) is not None:
                    ml.ant_debug = canon(ml.ant_debug)
    return nc


_NC_CACHE = {}
_DISK_DIR = "/tmp/bass_kernel_cache"


class _NcShim:
    """Duck-typed stand-in for the Bass object on run_bass_kernel_spmd's axon
    path: exposes the compiled module plus the handful of attributes the
    bass2jax lowering reads, with to_json_bytes() returning the cached
    serialization (skips re-serializing the module on every call, and lets a
    fresh process skip the whole tile-framework build via the disk cache)."""

    target_bir_lowering = False
    partition_id_tensor = None
    dbg_addr = None
    debug = False
    dbg_callbacks = ()
    has_collectives = False

    class _PidT:
        name = "partition_id"

    def __init__(self, m, json_bytes):
        self.m = m
        self._json = json_bytes
        for alloc in m.functions[0].allocations:
            if (isinstance(alloc, mybir.MemoryLocationSet)
                    and alloc.kind == "ExternalInput"
                    and alloc.memorylocations
                    and alloc.memorylocations[0].name == "partition_id"):
                self.partition_id_tensor = self._PidT()
                break

    def to_json_bytes(self):
        return self._json


def _get_nc(key, inputs):
    if key in _NC_CACHE:
        return _NC_CACHE[key]
    path = os.path.join(_DISK_DIR, key + ".birj")
    shim = None
    if os.path.exists(path):
        try:
            j = open(path, "rb").read()
            shim = _NcShim(mybir.module_from_json_bytes(j), j)
        except Exception:
            shim = None
    if shim is None:
        nc = build_kernel(_marshal_weights(inputs))
        j = nc.to_json_bytes()
        shim = _NcShim(nc.m, j)
        try:
            os.makedirs(_DISK_DIR, exist_ok=True)
            tmp = f"{path}.tmp{os.getpid()}"
            with open(tmp, "wb") as f:
                f.write(j)
            os.replace(tmp, path)
        except Exception:
            pass
    _NC_CACHE[key] = shim
    return shim


_WEIGHT_KEYS = (
    "Wq_w", "Wq_b", "kctx_w", "ktgt_w", "kphi_w1", "kphi_b1", "kphi_w2",
    "kphi_b2", "vctx_w", "vtgt_w", "vphi_w1", "vphi_b1", "vphi_w2", "vphi_b2",
    "gate_w", "gate_b", "out_w", "out_b",
)


_KVER = b"hc-attn-v6-forloop2"  # bump when build_kernel's emitted program changes


def _whash(inputs):
    h = hashlib.blake2b(digest_size=16)
    h.update(_KVER)
    for k in _WEIGHT_KEYS:
        a = np.ascontiguousarray(np.asarray(inputs[k], np.float32))
        h.update(k.encode())
        h.update(str(a.shape).encode())
        h.update(a.tobytes())
    return h.hexdigest()


def _marshal_weights(inputs):
    f32 = np.float32
    gw = np.asarray(inputs["gate_w"], f32)
    wg1, wg2, wg3 = gw[0:256], gw[256:512], gw[512:768]
    kphi_w2 = np.asarray(inputs["kphi_w2"], f32)
    vphi_w2 = np.asarray(inputs["vphi_w2"], f32)
    sc = 1.0 / np.sqrt(DK)

    mask = np.zeros((256, H), f32)
    for d in range(256):
        mask[d, d // 32] = 1.0
    e_hd = np.ascontiguousarray(mask.T)
    mask_p = _pack(mask)

    wv = {
        "w1k_n": -np.asarray(inputs["kphi_w1"], f32),
        "w1v_n": -np.asarray(inputs["vphi_w1"], f32),
        "b1k": np.asarray(inputs["kphi_b1"], f32).reshape(HID, 1),
        "b1v": np.asarray(inputs["vphi_b1"], f32).reshape(HID, 1),
        "w2k": kphi_w2, "w2v": vphi_w2, "w2v_n": -vphi_w2,
        "kctx_w": _pack(np.asarray(inputs["kctx_w"], f32)),
        "vctx_w": _pack(np.asarray(inputs["vctx_w"], f32)),
        "dctx_w": _pack(np.asarray(inputs["kctx_w"], f32)
                        - np.asarray(inputs["vctx_w"], f32)),
        "wq_s": _pack(np.asarray(inputs["Wq_w"], f32) * sc),
        "bq_s": _packb(np.asarray(inputs["Wq_b"], f32) * sc),
        "ktgt_w": _pack(np.asarray(inputs["ktgt_w"], f32)),
        "vtgt_w": _pack(np.asarray(inputs["vtgt_w"], f32)),
        "dtgt_w": _pack(np.asarray(inputs["ktgt_w"], f32)
                        - np.asarray(inputs["vtgt_w"], f32)),
        "b2k": _packb(np.asarray(inputs["kphi_b2"], f32)),
        "b2v": _packb(np.asarray(inputs["vphi_b2"], f32)),
        "db2": _packb(np.asarray(inputs["kphi_b2"], f32)
                      - np.asarray(inputs["vphi_b2"], f32)),
        "wg1": _pack(wg1), "wg2": _pack(wg2), "wg3": _pack(wg3),
        "wkg1": np.ascontiguousarray(kphi_w2 @ wg1),
        "wvg2": np.ascontiguousarray(vphi_w2 @ wg2),
        "gate_b": _packb(np.asarray(inputs["gate_b"], f32)),
        "out_w": _pack(np.asarray(inputs["out_w"], f32)),
        "out_b": _packb(np.asarray(inputs["out_b"], f32)),
        "mask_qh": mask_p, "e_hd": e_hd, "ident": np.eye(128, dtype=f32),
    }
    return {k: np.ascontiguousarray(
                np.asarray(v, f32).astype(BF16NP) if k in R_NAMES
                else np.asarray(v, f32))
            for k, v in wv.items()}


def kernel(**inputs):
    f32 = np.float32
    # If the caller hands us device-backed (jax) arrays, fetch them all in
    # one batched async device_get -- per-array np.asarray would pay a full
    # relay round-trip each (and np.asarray(x, dtype) can even trigger a
    # device-side convert compile).
    if any(not isinstance(v, np.ndarray) for v in inputs.values()):
        inputs = jax.device_get(inputs)
    key = _whash(inputs)
    nc = _get_nc(key, inputs)

    R_t = np.asarray(inputs["R_t"], f32)
    R_ctx = np.asarray(inputs["R_ctx"], f32)
    phi_t = np.asarray(inputs["phi_t"], f32)
    phi_c = np.asarray(inputs["phi_c"], f32)

    in_maps = []
    for core in range(NCORES):
        b, hh = core // 2, core % 2
        tsl = slice(hh * NT, (hh + 1) * NT)
        rt_p = R_t[b, tsl].T.reshape(2, 128, NT).transpose(1, 0, 2)
        rctx_p = R_ctx[b].T.reshape(2, 128, NC).transpose(1, 0, 2)
        act = np.ascontiguousarray(
            np.concatenate([rt_p, rctx_p], axis=2).astype(BF16NP))
        phi = np.ascontiguousarray(
            np.concatenate([phi_t[b, tsl].T, phi_c[b].T], axis=1))
        in_maps.append({"act": act, "phi": phi})

    res = run_bass_kernel_spmd(nc, in_maps, core_ids=list(range(NCORES)))
    kernel.last_results = res

    out = np.empty((B, NT_FULL, D), f32)
    for core in range(NCORES):
        r = res.results[core]["out_t"]            # (128, 2, 64) bf16
        arr = r.transpose(2, 1, 0).reshape(NT, D)
        b, hh = core // 2, core % 2
        out[b, hh * NT:(hh + 1) * NT, :] = arr.astype(f32)
    return out
